# revision 17
# baseline (speedup 1.0000x reference)
"""Trainium2 Bass kernel for nn_AlphaEntmax (entmax-bisect over last axis).

Key math fact: the module's ClampMin/ClampMax composition maps any alpha in
[1,2] to exactly 2.0, so the reference computes sparsemax (alpha=2) per row:
    p = relu(x - tau) / sum(relu(x - tau)),  tau s.t. sum(relu(x - tau)) = 1
We solve for tau with 4 over-relaxed Newton steps from tau0 = rowmax - 0.8
(tau' = tau + lam*(r-1)/c with r = sum(relu(x-tau)), c = count(x > tau),
lam = [1.15, 1.10, 1.05, 1.0]), then emit p = relu(x - tau) directly (sum==1
at convergence; the reference's own normalize brings both within tolerance).

Engine split (per [128,1024] tile, 64 tiles per core):
  - r-passes need no relu op: r = Sx - sum(min(x, tau)) with Sx = sum(x)
    computed once, so each r-pass is a plain VectorE tensor_scalar(op0=min,
    accum=add) on the fp16 shadow in 4x DVE mode (~300ns). The min-sum's
    magnitude is ~|Sx| ~ 30 (the inactive elements keep their raw values,
    mean 0), so the hardware f32 accumulator drift is ~1e-5. The naive
    dual  sum(max(x,tau)) = K*tau + r  looked cheaper (no Sx pass) but its
    accumulated magnitude ~2500 costs ~5e-2 of drift on HW — unusable.
  - c-passes: VectorE is_gt (fp16 4x); one slot per group is staggered onto
    ScalarE via ACT Sign(scale=-1, bias=tau): acc = K - 2c; and slot 0 of
    one group per wave can go to GPSIMD (tensor_scalar is_gt).
  - fp16 shadow (not bf16): 8x lower quantization floor, same 4x speed.
  - final pass on ScalarE: ACT Relu(x_f32 + ntau), written in place over
    the f32 x tile, stored from the Pool SWDGE queue (loads use SP) so
    loads and stores never head-of-line block each other.
  - stats live in one [128, 9*12] tile per 12-tile wave: each per-slot
    update chain is ~6 small VectorE ops for the whole wave.
DMA is the roofline: 32MB in + 32MB out per core over ~360GB/s ≈ 186us.

Sharding: x [8,16,512,1024] is split along the batch axis, one batch entry
(8192 rows of 1024) per NeuronCore; no cross-core communication.
"""

import numpy as np

B, H, Q, K = 8, 16, 512, 1024
N_CORES = 8
P = 128
ROWS_PER_CORE = (B // N_CORES) * H * Q  # 8192
N_TILES = ROWS_PER_CORE // P  # 64
G = 4  # tiles per group (DMA/cast granularity)
N_GROUPS = N_TILES // G  # 16
N_SLOTS = 4
D0 = 0.8  # tau0 = rowmax - D0
LAM = [1.15, 1.10, 1.05, 1.0]  # per-slot Newton over-relaxation
W = 3  # groups per wave; stats + update chains are per-wave
# count-pass engine rotation: at slot i, group (i mod W) counts on ScalarE
# (ACT Sign), POOL_CNT_TILES tiles of group ((i+1) mod W) on GPSIMD, rest V
POOL_CNT_TILES = 4  # tiles per Pool-assigned group that actually go to Pool
V_FINAL_KPOS = ()  # group positions whose final runs on VectorE (in-place)
UPDATE_MODE = "wave"  # "wave": one update chain per wave-slot; "group": per group
BUFS = {"xp": 9, "hp": 6, "st": 4}

_NC_CACHE = None


def _build_nc():
    import concourse.bacc as bacc
    import concourse.mybir as mybir
    from concourse.tile import TileContext

    f32 = mybir.dt.float32
    f16 = mybir.dt.float16
    Alu = mybir.AluOpType
    Act = mybir.ActivationFunctionType

    nc = bacc.Bacc(
        "TRN2", target_bir_lowering=False, debug=False, num_devices=N_CORES
    )
    x_ext = nc.dram_tensor("x", [ROWS_PER_CORE, K], f32, kind="ExternalInput")
    out_ext = nc.dram_tensor("out", [ROWS_PER_CORE, K], f32, kind="ExternalOutput")

    waves = []
    g0 = 0
    while g0 < N_GROUPS:
        waves.append(list(range(g0, min(g0 + W, N_GROUPS))))
        g0 += W
    NW = len(waves)

    ST_NAMES = ("mx", "tau", "ntau", "sx", "acc", "cacc", "r", "c", "rcp", "stp")

    with TileContext(nc) as tc:
        with (
            tc.tile_pool(name="xp", bufs=BUFS["xp"]) as xp,
            tc.tile_pool(name="hp", bufs=BUFS["hp"]) as hp,
            tc.tile_pool(name="scr", bufs=1) as scr,
            tc.tile_pool(name="st", bufs=BUFS["st"]) as st,
        ):
            # engine-dedicated elementwise-output scratch (never read back)
            scrR = scr.tile([P, K], f16, tag="scrR")  # V r/Sx out
            scrC = scr.tile([P, K], f16, tag="scrC")  # V c-pass out
            scrS = scr.tile([P, K], f16, tag="scrS", name="scrS")  # S sign out
            scrP = scr.tile([P, K], f16, tag="scrP", name="scrP")  # Pool out

            # warm the ACT table (a set containing Sign AND Relu) so the
            # one-time ~2.7us table load overlaps the first DMA
            nc.scalar.activation(
                scrS[:, :1], nc.const_aps.aps[(f32, 0.0)], Act.Sign
            )
            nc.scalar.activation(
                scrS[:, 1:2], nc.const_aps.aps[(f32, 0.0)], Act.Relu
            )

            # ---- per-wave state ----
            xbs = {}   # g -> xb tile
            xhs = {}   # g -> xh tile
            wstt = {}  # w -> dict name -> AP over [P, n_cols(w)]
            wcols = {}  # w -> n tiles in wave

            def emit_wave_loads(w):
                for g in waves[w]:
                    rows = slice(g * G * P, (g + 1) * G * P)
                    x_dram = x_ext.ap()[rows, :].rearrange(
                        "(t p) k -> p t k", p=P
                    )
                    xb = xp.tile([P, G * K], f32, tag="xb")
                    nc.sync.dma_start(
                        out=xb[:].rearrange("p (t k) -> p t k", t=G), in_=x_dram
                    )
                    xbs[g] = xb

            def col(w, g, j):
                # stats column index for tile j of group g within wave w
                return waves[w].index(g) * G + j

            def alloc_wave_stats(w):
                ncols = len(waves[w]) * G
                wcols[w] = ncols
                st_t = st.tile([P, len(ST_NAMES) * ncols], f32, tag="st")
                wstt[w] = {
                    n: st_t[:, k * ncols : (k + 1) * ncols]
                    for k, n in enumerate(ST_NAMES)
                }

            def emit_cast_group(w, kpos):
                stt = wstt[w]
                g = waves[w][kpos]
                xh = hp.tile([P, G * K], f16, tag="xh")
                xhs[g] = xh
                xb = xbs[g]
                for j in range(G):
                    cidx = kpos * G + j
                    # fp16 shadow + exact row max via f32 accum (pre-cast)
                    nc.vector.tensor_scalar(
                        xh[:, j * K : (j + 1) * K],
                        xb[:, j * K : (j + 1) * K],
                        0.0, None, Alu.add, Alu.max,
                        accum_out=stt["mx"][:, cidx : cidx + 1],
                    )
                    # Sx = sum(xh)  (fp16 4x pass)
                    nc.vector.tensor_scalar(
                        scrR[:], xh[:, j * K : (j + 1) * K],
                        0.0, None, Alu.add, Alu.add,
                        accum_out=stt["sx"][:, cidx : cidx + 1],
                    )
                # tau0 = mx - D0
                sl = slice(kpos * G, (kpos + 1) * G)
                nc.vector.tensor_scalar(
                    stt["tau"][:, sl], stt["mx"][:, sl], -D0, None, Alu.add
                )

            def emit_wave_casts(w):
                alloc_wave_stats(w)
                for kpos in range(len(waves[w])):
                    emit_cast_group(w, kpos)

            def emit_group_mins(w, i, kpos):
                stt = wstt[w]
                g = waves[w][kpos]
                xh = xhs[g]
                for j in range(G):
                    cidx = kpos * G + j
                    # acc = sum(min(x, tau));  r = Sx - acc
                    nc.vector.tensor_scalar(
                        scrR[:], xh[:, j * K : (j + 1) * K],
                        stt["tau"][:, cidx : cidx + 1], None,
                        Alu.min, Alu.add,
                        accum_out=stt["acc"][:, cidx : cidx + 1],
                    )

            def emit_group_counts(w, i, kpos, eng):
                stt = wstt[w]
                g = waves[w][kpos]
                xh, xb = xhs[g], xbs[g]
                for j in range(G):
                    cidx = kpos * G + j
                    tau_j = stt["tau"][:, cidx : cidx + 1]
                    xhj = xh[:, j * K : (j + 1) * K]
                    if eng == "S":
                        # cacc = sum(sign(tau - x)) = K - 2c
                        nc.scalar.activation(
                            scrS[:], xb[:, j * K : (j + 1) * K], Act.Sign,
                            bias=tau_j, scale=-1.0,
                            accum_out=stt["cacc"][:, cidx : cidx + 1],
                        )
                    elif eng == "P" and j < POOL_CNT_TILES:
                        nc.gpsimd.tensor_scalar(
                            scrP[:], xhj, tau_j, None,
                            Alu.is_gt, Alu.add,
                            accum_out=stt["cacc"][:, cidx : cidx + 1],
                        )
                    else:
                        nc.vector.tensor_scalar(
                            scrC[:], xhj, tau_j, None,
                            Alu.is_gt, Alu.add,
                            accum_out=stt["cacc"][:, cidx : cidx + 1],
                        )

            def emit_update(w, i, sl, sign_sl):
                """Update chain over stats columns `sl`; `sign_sl` is the
                sub-slice whose counts came from ACT Sign (or None)."""
                stt = wstt[w]
                tau = stt["tau"][:, sl]
                r = stt["r"][:, sl]
                c = stt["c"][:, sl]
                # r = sx - acc
                nc.vector.tensor_tensor(
                    r[:], stt["sx"][:, sl], stt["acc"][:, sl], Alu.subtract
                )
                if sign_sl is not None:
                    # c = (K - cacc)/2 on the sign-sourced columns
                    nc.vector.tensor_scalar(
                        stt["cacc"][:, sign_sl], stt["cacc"][:, sign_sl],
                        -0.5, float(K) * 0.5, Alu.mult, Alu.add,
                    )
                nc.vector.tensor_scalar_max(c[:], stt["cacc"][:, sl], 1.0)
                nc.vector.reciprocal(stt["rcp"][:, sl], c[:])
                # stp = (r - 1) * rcp
                nc.vector.scalar_tensor_tensor(
                    stt["stp"][:, sl], r[:], -1.0, stt["rcp"][:, sl],
                    Alu.add, Alu.mult,
                )
                # tau += lam * stp
                nc.vector.scalar_tensor_tensor(
                    tau[:], stt["stp"][:, sl], float(LAM[i]), tau[:],
                    Alu.mult, Alu.add,
                )
                if i == N_SLOTS - 1:
                    nc.vector.tensor_scalar(
                        stt["ntau"][:, sl], tau[:], -1.0, None, Alu.mult
                    )

            def emit_wave_slot(w, i):
                nw = len(waves[w])
                s_grp = i % W if i % W < nw else -1
                p_grp = (i + 1) % W
                if p_grp >= nw or p_grp == s_grp:
                    p_grp = -1
                # cross-engine counts issued first so ScalarE/GPSIMD start
                # while VectorE chews its own passes
                if s_grp >= 0:
                    emit_group_counts(w, i, s_grp, "S")
                if p_grp >= 0:
                    emit_group_counts(w, i, p_grp, "P")
                # V-group counts last-updated groups first... V passes:
                order = [k for k in range(nw) if k not in (s_grp, p_grp)]
                order += [k for k in (p_grp, s_grp) if k >= 0]
                for kpos in order:
                    emit_group_mins(w, i, kpos)
                    if kpos not in (s_grp, p_grp):
                        emit_group_counts(w, i, kpos, "V")
                    elif kpos == p_grp:
                        stt = wstt[w]
                        g = waves[w][kpos]
                        for j in range(POOL_CNT_TILES, G):
                            cidx = kpos * G + j
                            nc.vector.tensor_scalar(
                                scrC[:], xhs[g][:, j * K : (j + 1) * K],
                                stt["tau"][:, cidx : cidx + 1], None,
                                Alu.is_gt, Alu.add,
                                accum_out=stt["cacc"][:, cidx : cidx + 1],
                            )
                    if UPDATE_MODE == "group":
                        sl = slice(kpos * G, (kpos + 1) * G)
                        emit_update(w, i, sl, sl if kpos == s_grp else None)
                if UPDATE_MODE == "wave":
                    sign_sl = (
                        slice(s_grp * G, (s_grp + 1) * G) if s_grp >= 0 else None
                    )
                    emit_update(w, i, slice(0, nw * G), sign_sl)

            def emit_wave_finals(w):
                stt = wstt[w]
                for kpos, g in enumerate(waves[w]):
                    rows = slice(g * G * P, (g + 1) * G * P)
                    o_dram = out_ext.ap()[rows, :].rearrange(
                        "(t p) k -> p t k", p=P
                    )
                    xb = xbs[g]
                    on_v = kpos in V_FINAL_KPOS
                    for j in range(G):
                        cidx = col(w, g, j)
                        xbj = xb[:, j * K : (j + 1) * K]
                        if on_v:
                            # in-place relu(x - tau) on VectorE (f32 2x)
                            nc.vector.tensor_scalar(
                                xbj, xbj, stt["tau"][:, cidx : cidx + 1], 0.0,
                                Alu.subtract, Alu.max,
                            )
                        else:
                            nc.scalar.activation(
                                xbj, xbj, Act.Relu,
                                bias=stt["ntau"][:, cidx : cidx + 1],
                            )
                    # stores ride the Pool SWDGE queue; loads use SP
                    nc.gpsimd.dma_start(
                        out=o_dram, in_=xb[:].rearrange("p (t k) -> p t k", t=G)
                    )

            emit_wave_loads(0)
            emit_wave_casts(0)
            if NW > 1:
                emit_wave_loads(1)
            for w in range(NW):
                if w + 2 < NW:
                    emit_wave_loads(w + 2)
                if w + 1 < NW:
                    alloc_wave_stats(w + 1)
                emit_wave_slot(w, 0)
                for i in range(1, N_SLOTS):
                    # spread next wave's casts between this wave's slots
                    if w + 1 < NW and i - 1 < len(waves[w + 1]):
                        emit_cast_group(w + 1, i - 1)
                    emit_wave_slot(w, i)
                if w + 1 < NW:
                    for kpos in range(N_SLOTS - 1, len(waves[w + 1])):
                        emit_cast_group(w + 1, kpos)
                emit_wave_finals(w)

    nc.compile()
    return nc


def _get_nc():
    global _NC_CACHE
    if _NC_CACHE is None:
        _NC_CACHE = _build_nc()
    return _NC_CACHE


def _effective_alpha(alpha):
    # the module's ClampMin/ClampMax pair, verbatim in numpy
    a = np.asarray(alpha, dtype=np.float32)
    a = np.maximum(np.minimum(a, 0.0) - 1.0, 0.0) + 1.0 + np.maximum(a, 0.0)
    a = np.minimum(np.maximum(a, 0.0) - 2.0, 0.0) + 2.0 + np.minimum(a, 0.0)
    return a.astype(np.float32)


def _entmax_bisect_numpy(x, a, n_iter=50):
    """Generic-alpha fallback replicating the reference bisection in f32.
    Never taken for alpha in [1,2] (the clamp maps those to exactly 2.0)."""
    f32 = np.float32
    X = x.reshape(-1, K).astype(np.float32)
    am1 = (np.broadcast_to(a.reshape(1, H), (B, H)).reshape(-1)[
        np.arange(X.shape[0]) // Q
    ].astype(np.float32) - f32(1.0))[:, None]
    Xs = (X * am1).astype(np.float32)

    def p(s):
        pos = s > 0
        return np.where(
            pos, np.power(np.where(pos, s, f32(1.0)), (f32(1.0) / am1)), f32(0.0)
        ).astype(np.float32)

    mx = Xs.max(axis=1, keepdims=True).astype(np.float32)
    tau_lo = (mx - f32(1.0)).astype(np.float32)
    tau_hi = (mx - np.power(f32(1.0 / K), am1)).astype(np.float32)
    f_lo = (p(Xs - tau_lo).sum(axis=1, dtype=np.float32)[:, None] - f32(1.0)).astype(
        np.float32
    )
    dm = (tau_hi - tau_lo).astype(np.float32)
    tau_m = tau_lo.copy()
    for _ in range(n_iter):
        dm = (dm * f32(0.5)).astype(np.float32)
        tau_m = (tau_lo + dm).astype(np.float32)
        f_m = (p(Xs - tau_m).sum(axis=1, dtype=np.float32)[:, None] - f32(1.0)).astype(
            np.float32
        )
        tau_lo = np.where(f_m * f_lo >= 0, tau_m, tau_lo).astype(np.float32)
    pm = p(Xs - tau_m)
    s = pm.sum(axis=1, dtype=np.float32).astype(np.float32)[:, None]
    return (pm / s).astype(np.float32).reshape(B, H, Q, K)


def kernel(**inputs) -> np.ndarray:
    from concourse.bass_utils import run_bass_kernel_spmd

    x = np.ascontiguousarray(np.asarray(inputs["x"], dtype=np.float32))
    alpha = np.asarray(inputs.get("alpha", np.full((1, H), 1.5, np.float32)))
    a_eff = _effective_alpha(alpha)
    if not np.all(a_eff == np.float32(2.0)):
        # out-of-distribution alpha (outside [1,2]): generic slow path
        return _entmax_bisect_numpy(x, a_eff)

    shards = x.reshape(N_CORES, ROWS_PER_CORE, K)
    in_maps = [{"x": shards[i]} for i in range(N_CORES)]

    nc = _get_nc()
    res = run_bass_kernel_spmd(nc, in_maps, core_ids=list(range(N_CORES)))
    out = np.stack([res.results[i]["out"] for i in range(N_CORES)])
    return out.reshape(B, H, Q, K)


# revision 21
# speedup vs baseline: 1.0561x; 1.0561x over previous
"""Trainium2 Bass kernel for nn_AlphaEntmax (entmax-bisect over last axis).

Key math fact: the module's ClampMin/ClampMax composition maps any alpha in
[1,2] to exactly 2.0, so the reference computes sparsemax (alpha=2) per row:
    p = relu(x - tau) / sum(relu(x - tau)),  tau s.t. sum(relu(x - tau)) = 1
We solve for tau with 4 over-relaxed Newton steps from tau0 = rowmax - 0.8
(tau' = tau + lam*(r-1)/c with r = sum(relu(x-tau)), c = count(x > tau),
lam = [1.15, 1.10, 1.05, 1.0]), then emit p = relu(x - tau) directly (sum==1
at convergence; the reference's own normalize brings both within tolerance).

Engine split (per [128,1024] tile, 64 tiles per core):
  - r-passes need no relu op: r = Sx - sum(min(x, tau)) with Sx = sum(x)
    computed once, so each r-pass is a plain VectorE tensor_scalar(op0=min,
    accum=add) on the fp16 shadow in 4x DVE mode (~300ns). The min-sum's
    magnitude is ~|Sx| ~ 30 (the inactive elements keep their raw values,
    mean 0), so the hardware f32 accumulator drift is ~1e-5. The naive
    dual  sum(max(x,tau)) = K*tau + r  looked cheaper (no Sx pass) but its
    accumulated magnitude ~2500 costs ~5e-2 of drift on HW — unusable.
  - c-passes: VectorE is_gt (fp16 4x); one slot per group is staggered onto
    ScalarE via ACT Sign(scale=-1, bias=tau): acc = K - 2c; and slot 0 of
    one group per wave can go to GPSIMD (tensor_scalar is_gt).
  - fp16 shadow (not bf16): 8x lower quantization floor, same 4x speed.
  - final pass on ScalarE: ACT Relu(x_f32 + ntau), written in place over
    the f32 x tile, stored from the Pool SWDGE queue (loads use SP) so
    loads and stores never head-of-line block each other.
  - stats live in one [128, 9*12] tile per 12-tile wave: each per-slot
    update chain is ~6 small VectorE ops for the whole wave.
DMA is the roofline: 32MB in + 32MB out per core over ~360GB/s ≈ 186us.

Sharding: x [8,16,512,1024] is split along the batch axis, one batch entry
(8192 rows of 1024) per NeuronCore; no cross-core communication.
"""

import numpy as np

B, H, Q, K = 8, 16, 512, 1024
N_CORES = 8
P = 128
ROWS_PER_CORE = (B // N_CORES) * H * Q  # 8192
N_TILES = ROWS_PER_CORE // P  # 64
G = 4  # tiles per group (DMA/cast granularity)
N_GROUPS = N_TILES // G  # 16
N_SLOTS = 4
D0 = 0.8  # tau0 = rowmax - D0
LAM = [1.15, 1.10, 1.05, 1.0]  # per-slot Newton over-relaxation
W = 3  # groups per wave; stats + update chains are per-wave
# count-pass engine rotation: at slot i, group (i mod W) counts on ScalarE
# (ACT Sign), POOL_CNT_TILES tiles of group ((i+1) mod W) on GPSIMD, rest V
POOL_CNT_TILES = 4  # tiles per Pool-assigned group that actually go to Pool
V_FINAL_KPOS = ()  # group positions whose final runs on VectorE (in-place)
UPDATE_MODE = "wave"  # "wave": one update chain per wave-slot; "group": per group
BUFS = {"xp": 9, "hp": 6, "st": 4}

_NC_CACHE = None


def _build_nc():
    import concourse.bacc as bacc
    import concourse.mybir as mybir
    from concourse.tile import TileContext

    f32 = mybir.dt.float32
    f16 = mybir.dt.float16
    Alu = mybir.AluOpType
    Act = mybir.ActivationFunctionType

    nc = bacc.Bacc(
        "TRN2", target_bir_lowering=False, debug=False, num_devices=N_CORES
    )
    x_ext = nc.dram_tensor("x", [ROWS_PER_CORE, K], f32, kind="ExternalInput")
    out_ext = nc.dram_tensor("out", [ROWS_PER_CORE, K], f32, kind="ExternalOutput")

    waves = []
    g0 = 0
    while g0 < N_GROUPS:
        waves.append(list(range(g0, min(g0 + W, N_GROUPS))))
        g0 += W
    NW = len(waves)

    ST_NAMES = ("mx", "tau", "ntau", "sx", "acc", "cacc", "r", "c", "rcp", "stp")

    with TileContext(nc) as tc:
        with (
            tc.tile_pool(name="xp", bufs=BUFS["xp"]) as xp,
            tc.tile_pool(name="hp", bufs=BUFS["hp"]) as hp,
            tc.tile_pool(name="scr", bufs=1) as scr,
            tc.tile_pool(name="st", bufs=BUFS["st"]) as st,
        ):
            # engine-dedicated elementwise-output scratch (never read back)
            scrR = scr.tile([P, K], f16, tag="scrR")  # V r/Sx out
            scrC = scr.tile([P, K], f16, tag="scrC")  # V c-pass out
            scrS = scr.tile([P, K], f16, tag="scrS", name="scrS")  # S sign out
            scrP = scr.tile([P, K], f16, tag="scrP", name="scrP")  # Pool out

            # warm the ACT table (a set containing Sign AND Relu) so the
            # one-time ~2.7us table load overlaps the first DMA
            nc.scalar.activation(
                scrS[:, :1], nc.const_aps.aps[(f32, 0.0)], Act.Sign
            )
            nc.scalar.activation(
                scrS[:, 1:2], nc.const_aps.aps[(f32, 0.0)], Act.Relu
            )

            # ---- per-wave state ----
            xbs = {}   # g -> xb tile
            xhs = {}   # g -> xh tile
            wstt = {}  # w -> dict name -> AP over [P, n_cols(w)]
            wcols = {}  # w -> n tiles in wave

            def emit_wave_loads(w, split_first=False):
                for gi, g in enumerate(waves[w]):
                    rows = slice(g * G * P, (g + 1) * G * P)
                    x_dram = x_ext.ap()[rows, :].rearrange(
                        "(t p) k -> p t k", p=P
                    )
                    xb = xp.tile([P, G * K], f32, tag="xb")
                    if split_first and gi == 0:
                        # per-tile loads so the first cast starts ~4x sooner
                        for j in range(G):
                            nc.sync.dma_start(
                                out=xb[:, j * K : (j + 1) * K],
                                in_=x_dram[:, j, :],
                            )
                    else:
                        nc.sync.dma_start(
                            out=xb[:].rearrange("p (t k) -> p t k", t=G),
                            in_=x_dram,
                        )
                    xbs[g] = xb

            def col(w, g, j):
                # stats column index for tile j of group g within wave w
                return waves[w].index(g) * G + j

            def alloc_wave_stats(w):
                ncols = len(waves[w]) * G
                wcols[w] = ncols
                st_t = st.tile([P, len(ST_NAMES) * ncols], f32, tag="st")
                wstt[w] = {
                    n: st_t[:, k * ncols : (k + 1) * ncols]
                    for k, n in enumerate(ST_NAMES)
                }

            def emit_cast_group(w, kpos):
                stt = wstt[w]
                g = waves[w][kpos]
                xh = hp.tile([P, G * K], f16, tag="xh")
                xhs[g] = xh
                xb = xbs[g]
                for j in range(G):
                    cidx = kpos * G + j
                    # fp16 shadow + exact row max via f32 accum (pre-cast)
                    nc.vector.tensor_scalar(
                        xh[:, j * K : (j + 1) * K],
                        xb[:, j * K : (j + 1) * K],
                        0.0, None, Alu.add, Alu.max,
                        accum_out=stt["mx"][:, cidx : cidx + 1],
                    )
                    # Sx = sum(xh)  (fp16 4x pass)
                    nc.vector.tensor_scalar(
                        scrR[:], xh[:, j * K : (j + 1) * K],
                        0.0, None, Alu.add, Alu.add,
                        accum_out=stt["sx"][:, cidx : cidx + 1],
                    )
                # tau0 = mx - D0
                sl = slice(kpos * G, (kpos + 1) * G)
                nc.vector.tensor_scalar(
                    stt["tau"][:, sl], stt["mx"][:, sl], -D0, None, Alu.add
                )

            def emit_wave_casts(w):
                alloc_wave_stats(w)
                for kpos in range(len(waves[w])):
                    emit_cast_group(w, kpos)

            def emit_group_mins(w, i, kpos):
                stt = wstt[w]
                g = waves[w][kpos]
                xh = xhs[g]
                for j in range(G):
                    cidx = kpos * G + j
                    # acc = sum(min(x, tau));  r = Sx - acc
                    nc.vector.tensor_scalar(
                        scrR[:], xh[:, j * K : (j + 1) * K],
                        stt["tau"][:, cidx : cidx + 1], None,
                        Alu.min, Alu.add,
                        accum_out=stt["acc"][:, cidx : cidx + 1],
                    )

            def emit_group_counts(w, i, kpos, eng):
                stt = wstt[w]
                g = waves[w][kpos]
                xh, xb = xhs[g], xbs[g]
                for j in range(G):
                    cidx = kpos * G + j
                    tau_j = stt["tau"][:, cidx : cidx + 1]
                    xhj = xh[:, j * K : (j + 1) * K]
                    if eng == "S":
                        # cacc = sum(sign(tau - x)) = K - 2c
                        nc.scalar.activation(
                            scrS[:], xb[:, j * K : (j + 1) * K], Act.Sign,
                            bias=tau_j, scale=-1.0,
                            accum_out=stt["cacc"][:, cidx : cidx + 1],
                        )
                    elif eng == "P" and j < POOL_CNT_TILES:
                        nc.gpsimd.tensor_scalar(
                            scrP[:], xhj, tau_j, None,
                            Alu.is_gt, Alu.add,
                            accum_out=stt["cacc"][:, cidx : cidx + 1],
                        )
                    else:
                        nc.vector.tensor_scalar(
                            scrC[:], xhj, tau_j, None,
                            Alu.is_gt, Alu.add,
                            accum_out=stt["cacc"][:, cidx : cidx + 1],
                        )

            def emit_update(w, i, sl, sign_sl):
                """Update chain over stats columns `sl`; `sign_sl` is the
                sub-slice whose counts came from ACT Sign (or None)."""
                stt = wstt[w]
                tau = stt["tau"][:, sl]
                r = stt["r"][:, sl]
                c = stt["c"][:, sl]
                # r = sx - acc
                nc.vector.tensor_tensor(
                    r[:], stt["sx"][:, sl], stt["acc"][:, sl], Alu.subtract
                )
                if sign_sl is not None:
                    # c = (K - cacc)/2 on the sign-sourced columns
                    nc.vector.tensor_scalar(
                        stt["cacc"][:, sign_sl], stt["cacc"][:, sign_sl],
                        -0.5, float(K) * 0.5, Alu.mult, Alu.add,
                    )
                nc.vector.tensor_scalar_max(c[:], stt["cacc"][:, sl], 1.0)
                nc.vector.reciprocal(stt["rcp"][:, sl], c[:])
                # stp = (r - 1) * rcp
                nc.vector.scalar_tensor_tensor(
                    stt["stp"][:, sl], r[:], -1.0, stt["rcp"][:, sl],
                    Alu.add, Alu.mult,
                )
                # tau += lam * stp
                nc.vector.scalar_tensor_tensor(
                    tau[:], stt["stp"][:, sl], float(LAM[i]), tau[:],
                    Alu.mult, Alu.add,
                )
                if i == N_SLOTS - 1:
                    nc.vector.tensor_scalar(
                        stt["ntau"][:, sl], tau[:], -1.0, None, Alu.mult
                    )

            def emit_wave_slot(w, i):
                nw = len(waves[w])
                if w == NW - 1:
                    # last wave: no overlap partner left — keep every count on
                    # VectorE so slot latency is V-local, not cross-engine
                    s_grp = p_grp = -1
                else:
                    s_grp = i % W if i % W < nw else -1
                    p_grp = (i + 1) % W
                    if p_grp >= nw or p_grp == s_grp:
                        p_grp = -1
                # cross-engine counts issued first so ScalarE/GPSIMD start
                # while VectorE chews its own passes
                if s_grp >= 0:
                    emit_group_counts(w, i, s_grp, "S")
                if p_grp >= 0:
                    emit_group_counts(w, i, p_grp, "P")
                # V-group counts last-updated groups first... V passes:
                order = [k for k in range(nw) if k not in (s_grp, p_grp)]
                order += [k for k in (p_grp, s_grp) if k >= 0]
                for kpos in order:
                    emit_group_mins(w, i, kpos)
                    if kpos not in (s_grp, p_grp):
                        emit_group_counts(w, i, kpos, "V")
                    elif kpos == p_grp:
                        stt = wstt[w]
                        g = waves[w][kpos]
                        for j in range(POOL_CNT_TILES, G):
                            cidx = kpos * G + j
                            nc.vector.tensor_scalar(
                                scrC[:], xhs[g][:, j * K : (j + 1) * K],
                                stt["tau"][:, cidx : cidx + 1], None,
                                Alu.is_gt, Alu.add,
                                accum_out=stt["cacc"][:, cidx : cidx + 1],
                            )
                    if UPDATE_MODE == "group":
                        sl = slice(kpos * G, (kpos + 1) * G)
                        emit_update(w, i, sl, sl if kpos == s_grp else None)
                if UPDATE_MODE == "wave":
                    sign_sl = (
                        slice(s_grp * G, (s_grp + 1) * G) if s_grp >= 0 else None
                    )
                    emit_update(w, i, slice(0, nw * G), sign_sl)

            def emit_wave_finals(w):
                stt = wstt[w]
                last = w == NW - 1
                for kpos, g in enumerate(waves[w]):
                    rows = slice(g * G * P, (g + 1) * G * P)
                    o_dram = out_ext.ap()[rows, :].rearrange(
                        "(t p) k -> p t k", p=P
                    )
                    xb = xbs[g]
                    on_v = last or kpos in V_FINAL_KPOS
                    half = G // 2 if last else G
                    for j in range(G):
                        cidx = col(w, g, j)
                        xbj = xb[:, j * K : (j + 1) * K]
                        if on_v:
                            # in-place relu(x - tau) on VectorE (f32 2x)
                            nc.vector.tensor_scalar(
                                xbj, xbj, stt["tau"][:, cidx : cidx + 1], 0.0,
                                Alu.subtract, Alu.max,
                            )
                        else:
                            nc.scalar.activation(
                                xbj, xbj, Act.Relu,
                                bias=stt["ntau"][:, cidx : cidx + 1],
                            )
                        if last and j + 1 == half:
                            # drain the first half-store while the second
                            # half's finals still run — shortens the tail
                            nc.gpsimd.dma_start(
                                out=o_dram[:, :half, :],
                                in_=xb[:, : half * K].rearrange(
                                    "p (t k) -> p t k", t=half
                                ),
                            )
                    # stores ride the Pool SWDGE queue; loads use SP
                    if last:
                        nc.gpsimd.dma_start(
                            out=o_dram[:, half:, :],
                            in_=xb[:, half * K :].rearrange(
                                "p (t k) -> p t k", t=G - half
                            ),
                        )
                    else:
                        nc.gpsimd.dma_start(
                            out=o_dram,
                            in_=xb[:].rearrange("p (t k) -> p t k", t=G),
                        )

            emit_wave_loads(0, split_first=True)
            emit_wave_casts(0)
            if NW > 1:
                emit_wave_loads(1)
            for w in range(NW):
                if w + 2 < NW:
                    emit_wave_loads(w + 2)
                if w + 1 < NW:
                    alloc_wave_stats(w + 1)
                emit_wave_slot(w, 0)
                for i in range(1, N_SLOTS):
                    # spread next wave's casts between this wave's slots
                    if w + 1 < NW and i - 1 < len(waves[w + 1]):
                        emit_cast_group(w + 1, i - 1)
                    emit_wave_slot(w, i)
                if w + 1 < NW:
                    for kpos in range(N_SLOTS - 1, len(waves[w + 1])):
                        emit_cast_group(w + 1, kpos)
                emit_wave_finals(w)

    nc.compile()
    return nc


def _get_nc():
    global _NC_CACHE
    if _NC_CACHE is None:
        _NC_CACHE = _build_nc()
    return _NC_CACHE


def _effective_alpha(alpha):
    # the module's ClampMin/ClampMax pair, verbatim in numpy
    a = np.asarray(alpha, dtype=np.float32)
    a = np.maximum(np.minimum(a, 0.0) - 1.0, 0.0) + 1.0 + np.maximum(a, 0.0)
    a = np.minimum(np.maximum(a, 0.0) - 2.0, 0.0) + 2.0 + np.minimum(a, 0.0)
    return a.astype(np.float32)


def _entmax_bisect_numpy(x, a, n_iter=50):
    """Generic-alpha fallback replicating the reference bisection in f32.
    Never taken for alpha in [1,2] (the clamp maps those to exactly 2.0)."""
    f32 = np.float32
    X = x.reshape(-1, K).astype(np.float32)
    am1 = (np.broadcast_to(a.reshape(1, H), (B, H)).reshape(-1)[
        np.arange(X.shape[0]) // Q
    ].astype(np.float32) - f32(1.0))[:, None]
    Xs = (X * am1).astype(np.float32)

    def p(s):
        pos = s > 0
        return np.where(
            pos, np.power(np.where(pos, s, f32(1.0)), (f32(1.0) / am1)), f32(0.0)
        ).astype(np.float32)

    mx = Xs.max(axis=1, keepdims=True).astype(np.float32)
    tau_lo = (mx - f32(1.0)).astype(np.float32)
    tau_hi = (mx - np.power(f32(1.0 / K), am1)).astype(np.float32)
    f_lo = (p(Xs - tau_lo).sum(axis=1, dtype=np.float32)[:, None] - f32(1.0)).astype(
        np.float32
    )
    dm = (tau_hi - tau_lo).astype(np.float32)
    tau_m = tau_lo.copy()
    for _ in range(n_iter):
        dm = (dm * f32(0.5)).astype(np.float32)
        tau_m = (tau_lo + dm).astype(np.float32)
        f_m = (p(Xs - tau_m).sum(axis=1, dtype=np.float32)[:, None] - f32(1.0)).astype(
            np.float32
        )
        tau_lo = np.where(f_m * f_lo >= 0, tau_m, tau_lo).astype(np.float32)
    pm = p(Xs - tau_m)
    s = pm.sum(axis=1, dtype=np.float32).astype(np.float32)[:, None]
    return (pm / s).astype(np.float32).reshape(B, H, Q, K)


def kernel(**inputs) -> np.ndarray:
    from concourse.bass_utils import run_bass_kernel_spmd

    x = np.ascontiguousarray(np.asarray(inputs["x"], dtype=np.float32))
    alpha = np.asarray(inputs.get("alpha", np.full((1, H), 1.5, np.float32)))
    a_eff = _effective_alpha(alpha)
    if not np.all(a_eff == np.float32(2.0)):
        # out-of-distribution alpha (outside [1,2]): generic slow path
        return _entmax_bisect_numpy(x, a_eff)

    shards = x.reshape(N_CORES, ROWS_PER_CORE, K)
    in_maps = [{"x": shards[i]} for i in range(N_CORES)]

    nc = _get_nc()
    res = run_bass_kernel_spmd(nc, in_maps, core_ids=list(range(N_CORES)))
    out = np.stack([res.results[i]["out"] for i in range(N_CORES)])
    return out.reshape(B, H, Q, K)


# revision 22
# speedup vs baseline: 1.0657x; 1.0091x over previous
"""Trainium2 Bass kernel for nn_AlphaEntmax (entmax-bisect over last axis).

Key math fact: the module's ClampMin/ClampMax composition maps any alpha in
[1,2] to exactly 2.0, so the reference computes sparsemax (alpha=2) per row:
    p = relu(x - tau) / sum(relu(x - tau)),  tau s.t. sum(relu(x - tau)) = 1
We solve for tau with 4 over-relaxed Newton steps from tau0 = rowmax - 0.8
(tau' = tau + lam*(r-1)/c with r = sum(relu(x-tau)), c = count(x > tau),
lam = [1.15, 1.10, 1.05, 1.0]), then emit p = relu(x - tau) from the fp16
shadow (sum==1 at convergence; the reference's own normalize brings both
within tolerance; fp16 sourcing costs ~2e-3 of the 2e-2 budget).

Architecture: OWNERSHIP LANES — each engine owns whole tiles end-to-end so
no per-slot cross-engine barrier exists (GPSIMD cannot run compute: the
neuronxcc ISA check rejects TensorScalarPtr on Pool).
  - VectorE-owned tiles (48): per slot a min-pass  acc = sum(min(x,tau))
    (tensor_scalar op0=min accum=add, fp16 4x mode ~300ns) and an is_gt
    count pass. r = Sx - acc with Sx = sum(x) computed once per tile: the
    min-sum's magnitude is ~|Sx| ~ 30, so the HW f32 accumulator drift is
    ~1e-5. (The cheaper-looking dual  sum(max(x,tau)) = K*tau + r  has
    accumulated magnitude ~2500 and drifts ~5e-2 on HW — unusable.)
  - ScalarE-owned tiles (16): ACT Relu(x + ntau, accum) gives r directly
    (relu terms are tiny, no drift) and ACT Sign(x + ntau, accum) gives
    K - 2c; iterates in ntau space. ACT is dtype-independent so it reads
    the fp16 shadow; ~1.1us per pass.
  - the fp16 (not bf16) shadow keeps the quantization floor ~8x lower at
    the same 4x DVE speed. All update chains are small [128,4] VectorE ops.
  - finals: owner engine emits relu(x_h - tau) to an f32 buffer; stores
    ride the Pool SWDGE queue, loads the SP queue (no head-of-line mixing).
  - emission: every lane unit gets a virtual-time stamp (its serial cost
    on the owning engine); units from all lanes are merged in stamp order
    so the in-order engines always interleave across tiles; loads are
    stamped ~15us before their tile's first use.
DMA is the roofline: 32MB in + 32MB out per core over ~360GB/s ≈ 186us.

Sharding: x [8,16,512,1024] is split along the batch axis, one batch entry
(8192 rows of 1024) per NeuronCore; no cross-core communication.
"""

import numpy as np

B, H, Q, K = 8, 16, 512, 1024
N_CORES = 8
P = 128
ROWS_PER_CORE = (B // N_CORES) * H * Q  # 8192
N_TILES = ROWS_PER_CORE // P  # 64
G = 4  # tiles per group (DMA/stats granularity)
N_GROUPS = N_TILES // G  # 16
N_SLOTS = 4
D0 = 0.8  # tau0 = rowmax - D0
LAM = [1.15, 1.10, 1.05, 1.0]  # per-slot Newton over-relaxation
# group indices owned by ScalarE (spread through the run for even pacing)
S_GROUPS = (2, 6, 10, 14)
PREFETCH_NS = 16000.0  # how far ahead (virtual time) loads are stamped
BUFS = {"xp": 6, "hp": 7, "op": 3, "st": 8}

# virtual-time unit costs (ns) — shape emission order only
VT = {
    "cast_v": 3900.0,   # 4 casts + 4 Sx + tau0
    "cast_s": 2600.0,   # 4 casts + ntau0
    "slot_v": 2800.0,   # 4 mins + 4 counts + update
    "slot_s": 9900.0,   # 4 ACT relu + 4 ACT sign + V update
    "final_v": 2700.0,  # 4 relu finals + store issue
    "final_s": 4800.0,
}

_NC_CACHE = None


def _build_nc():
    import concourse.bacc as bacc
    import concourse.mybir as mybir
    from concourse.tile import TileContext

    f32 = mybir.dt.float32
    f16 = mybir.dt.float16
    Alu = mybir.AluOpType
    Act = mybir.ActivationFunctionType

    nc = bacc.Bacc(
        "TRN2", target_bir_lowering=False, debug=False, num_devices=N_CORES
    )
    x_ext = nc.dram_tensor("x", [ROWS_PER_CORE, K], f32, kind="ExternalInput")
    out_ext = nc.dram_tensor("out", [ROWS_PER_CORE, K], f32, kind="ExternalOutput")

    ST_NAMES = ("mx", "tau", "sx", "acc", "cacc", "r", "c", "rcp", "stp")
    NST = len(ST_NAMES)

    with TileContext(nc) as tc:
        with (
            tc.tile_pool(name="xp", bufs=BUFS["xp"]) as xp,
            tc.tile_pool(name="hp", bufs=BUFS["hp"]) as hp,
            tc.tile_pool(name="op", bufs=BUFS["op"]) as op,
            tc.tile_pool(name="scr", bufs=1) as scr,
            tc.tile_pool(name="st", bufs=BUFS["st"]) as st,
        ):
            # engine-dedicated elementwise-output scratch (never read back)
            scrR = scr.tile([P, K], f16, tag="scrR")  # V min/Sx out
            scrC = scr.tile([P, K], f16, tag="scrC")  # V count out
            scrS = scr.tile([P, K], f16, tag="scrS", name="scrS")  # S out

            # warm the ACT table (a set containing Sign AND Relu) so the
            # one-time ~2.7us table load overlaps the first DMA
            nc.scalar.activation(
                scrS[:, :1], nc.const_aps.aps[(f32, 0.0)], Act.Sign
            )
            nc.scalar.activation(
                scrS[:, 1:2], nc.const_aps.aps[(f32, 0.0)], Act.Relu
            )

            xbs, xhs, stts = {}, {}, {}

            def emit_load(g):
                rows = slice(g * G * P, (g + 1) * G * P)
                x_dram = x_ext.ap()[rows, :].rearrange("(t p) k -> p t k", p=P)
                xb = xp.tile([P, G * K], f32, tag="xb")
                if g == 0:
                    # per-tile loads so the first cast starts ~4x sooner
                    for j in range(G):
                        nc.sync.dma_start(
                            out=xb[:, j * K : (j + 1) * K], in_=x_dram[:, j, :]
                        )
                else:
                    nc.sync.dma_start(
                        out=xb[:].rearrange("p (t k) -> p t k", t=G), in_=x_dram
                    )
                xbs[g] = xb

            def emit_cast(g):
                """fp16 shadow + row max; V-owned also get Sx; then the
                slot-0 iterate (tau for V-owned, ntau for S-owned)."""
                on_s = g in S_GROUPS
                xb = xbs.pop(g)
                xh = hp.tile([P, G * K], f16, tag="xh")
                xhs[g] = xh
                st_t = st.tile([P, NST * G], f32, tag="st")
                stt = {
                    n: st_t[:, k * G : (k + 1) * G]
                    for k, n in enumerate(ST_NAMES)
                }
                stts[g] = stt
                for j in range(G):
                    sl1 = slice(j, j + 1)
                    nc.vector.tensor_scalar(
                        xh[:, j * K : (j + 1) * K], xb[:, j * K : (j + 1) * K],
                        0.0, None, Alu.add, Alu.max,
                        accum_out=stt["mx"][:, sl1],
                    )
                    if not on_s:
                        nc.vector.tensor_scalar(
                            scrR[:], xh[:, j * K : (j + 1) * K],
                            0.0, None, Alu.add, Alu.add,
                            accum_out=stt["sx"][:, sl1],
                        )
                if on_s:
                    # ntau0 = D0 - mx   (ScalarE lane iterates in ntau space)
                    nc.vector.tensor_scalar(
                        stt["tau"][:], stt["mx"][:], -1.0, D0,
                        Alu.mult, Alu.add,
                    )
                else:
                    # tau0 = mx - D0
                    nc.vector.tensor_scalar(
                        stt["tau"][:], stt["mx"][:], -D0, None, Alu.add
                    )

            def emit_slot_v(g, i):
                stt = stts[g]
                xh = xhs[g]
                tau = stt["tau"]
                for j in range(G):
                    sl1 = slice(j, j + 1)
                    xhj = xh[:, j * K : (j + 1) * K]
                    # acc = sum(min(x, tau));  r = Sx - acc
                    nc.vector.tensor_scalar(
                        scrR[:], xhj, tau[:, sl1], None, Alu.min, Alu.add,
                        accum_out=stt["acc"][:, sl1],
                    )
                    nc.vector.tensor_scalar(
                        scrC[:], xhj, tau[:, sl1], None, Alu.is_gt, Alu.add,
                        accum_out=stt["cacc"][:, sl1],
                    )
                # ---- update chain ----
                nc.vector.tensor_tensor(
                    stt["r"][:], stt["sx"][:], stt["acc"][:], Alu.subtract
                )
                nc.vector.tensor_scalar_max(stt["c"][:], stt["cacc"][:], 1.0)
                nc.vector.reciprocal(stt["rcp"][:], stt["c"][:])
                nc.vector.scalar_tensor_tensor(
                    stt["stp"][:], stt["r"][:], -1.0, stt["rcp"][:],
                    Alu.add, Alu.mult,
                )
                nc.vector.scalar_tensor_tensor(
                    tau[:], stt["stp"][:], float(LAM[i]), tau[:],
                    Alu.mult, Alu.add,
                )

            def emit_slot_s(g, i):
                stt = stts[g]
                xh = xhs[g]
                ntau = stt["tau"]  # ntau = -tau for the ScalarE lane
                for j in range(G):
                    sl1 = slice(j, j + 1)
                    xhj = xh[:, j * K : (j + 1) * K]
                    # r = sum(relu(x + ntau))  — tiny terms, drift-free
                    nc.scalar.activation(
                        scrS[:], xhj, Act.Relu, bias=ntau[:, sl1],
                        accum_out=stt["r"][:, sl1],
                    )
                    # cacc = sum(sign(x + ntau)) = 2c - K
                    nc.scalar.activation(
                        scrS[:], xhj, Act.Sign, bias=ntau[:, sl1],
                        accum_out=stt["cacc"][:, sl1],
                    )
                # ---- update chain (VectorE) ----
                # c = (cacc + K)/2, guard >= 1
                nc.vector.tensor_scalar(
                    stt["c"][:], stt["cacc"][:], 0.5, float(K) * 0.5,
                    Alu.mult, Alu.add,
                )
                nc.vector.tensor_scalar_max(stt["c"][:], stt["c"][:], 1.0)
                nc.vector.reciprocal(stt["rcp"][:], stt["c"][:])
                nc.vector.scalar_tensor_tensor(
                    stt["stp"][:], stt["r"][:], -1.0, stt["rcp"][:],
                    Alu.add, Alu.mult,
                )
                # ntau -= lam * stp
                nc.vector.scalar_tensor_tensor(
                    ntau[:], stt["stp"][:], -float(LAM[i]), ntau[:],
                    Alu.mult, Alu.add,
                )

            def emit_final(g):
                on_s = g in S_GROUPS
                stt = stts.pop(g)
                xh = xhs.pop(g)
                rows = slice(g * G * P, (g + 1) * G * P)
                o_dram = out_ext.ap()[rows, :].rearrange("(t p) k -> p t k", p=P)
                ob = op.tile([P, G * K], f32, tag="ob")
                for j in range(G):
                    sl1 = slice(j, j + 1)
                    xhj = xh[:, j * K : (j + 1) * K]
                    obj = ob[:, j * K : (j + 1) * K]
                    if on_s:
                        nc.scalar.activation(
                            obj, xhj, Act.Relu, bias=stt["tau"][:, sl1]
                        )
                    else:
                        nc.vector.tensor_scalar(
                            obj, xhj, stt["tau"][:, sl1], 0.0,
                            Alu.subtract, Alu.max,
                        )
                nc.gpsimd.dma_start(
                    out=o_dram, in_=ob[:].rearrange("p (t k) -> p t k", t=G)
                )

            # ---- build the two lanes as (vtime, seq, emit_fn) events ----
            events = []
            seq = 0

            def push(t, fn):
                nonlocal seq
                events.append((t, seq, fn))
                seq += 1

            v_groups = [g for g in range(N_GROUPS) if g not in S_GROUPS]
            s_groups = [g for g in range(N_GROUPS) if g in S_GROUPS]

            cast_time = {}
            vt = 0.0
            for g in v_groups:
                cast_time[g] = vt
                push(vt, lambda g=g: emit_cast(g))
                vt += VT["cast_v"]
                for i in range(N_SLOTS):
                    push(vt, lambda g=g, i=i: emit_slot_v(g, i))
                    vt += VT["slot_v"]
                push(vt, lambda g=g: emit_final(g))
                vt += VT["final_v"]

            vt = 0.0
            for g in s_groups:
                cast_time[g] = vt
                # the S-lane's cast runs on V; stamp it just before use
                push(vt, lambda g=g: emit_cast(g))
                vt += VT["cast_s"]
                for i in range(N_SLOTS):
                    push(vt, lambda g=g, i=i: emit_slot_s(g, i))
                    vt += VT["slot_s"]
                push(vt, lambda g=g: emit_final(g))
                vt += VT["final_s"]

            for g in range(N_GROUPS):
                push(max(cast_time[g] - PREFETCH_NS, -1.0 - (N_GROUPS - g)),
                     lambda g=g: emit_load(g))

            events.sort(key=lambda e: (e[0], e[1]))
            for _, _, fn in events:
                fn()

    nc.compile()
    return nc


def _get_nc():
    global _NC_CACHE
    if _NC_CACHE is None:
        _NC_CACHE = _build_nc()
    return _NC_CACHE


def _effective_alpha(alpha):
    # the module's ClampMin/ClampMax pair, verbatim in numpy
    a = np.asarray(alpha, dtype=np.float32)
    a = np.maximum(np.minimum(a, 0.0) - 1.0, 0.0) + 1.0 + np.maximum(a, 0.0)
    a = np.minimum(np.maximum(a, 0.0) - 2.0, 0.0) + 2.0 + np.minimum(a, 0.0)
    return a.astype(np.float32)


def _entmax_bisect_numpy(x, a, n_iter=50):
    """Generic-alpha fallback replicating the reference bisection in f32.
    Never taken for alpha in [1,2] (the clamp maps those to exactly 2.0)."""
    f32 = np.float32
    X = x.reshape(-1, K).astype(np.float32)
    am1 = (np.broadcast_to(a.reshape(1, H), (B, H)).reshape(-1)[
        np.arange(X.shape[0]) // Q
    ].astype(np.float32) - f32(1.0))[:, None]
    Xs = (X * am1).astype(np.float32)

    def p(s):
        pos = s > 0
        return np.where(
            pos, np.power(np.where(pos, s, f32(1.0)), (f32(1.0) / am1)), f32(0.0)
        ).astype(np.float32)

    mx = Xs.max(axis=1, keepdims=True).astype(np.float32)
    tau_lo = (mx - f32(1.0)).astype(np.float32)
    tau_hi = (mx - np.power(f32(1.0 / K), am1)).astype(np.float32)
    f_lo = (p(Xs - tau_lo).sum(axis=1, dtype=np.float32)[:, None] - f32(1.0)).astype(
        np.float32
    )
    dm = (tau_hi - tau_lo).astype(np.float32)
    tau_m = tau_lo.copy()
    for _ in range(n_iter):
        dm = (dm * f32(0.5)).astype(np.float32)
        tau_m = (tau_lo + dm).astype(np.float32)
        f_m = (p(Xs - tau_m).sum(axis=1, dtype=np.float32)[:, None] - f32(1.0)).astype(
            np.float32
        )
        tau_lo = np.where(f_m * f_lo >= 0, tau_m, tau_lo).astype(np.float32)
    pm = p(Xs - tau_m)
    s = pm.sum(axis=1, dtype=np.float32).astype(np.float32)[:, None]
    return (pm / s).astype(np.float32).reshape(B, H, Q, K)


def kernel(**inputs) -> np.ndarray:
    from concourse.bass_utils import run_bass_kernel_spmd

    x = np.ascontiguousarray(np.asarray(inputs["x"], dtype=np.float32))
    alpha = np.asarray(inputs.get("alpha", np.full((1, H), 1.5, np.float32)))
    a_eff = _effective_alpha(alpha)
    if not np.all(a_eff == np.float32(2.0)):
        # out-of-distribution alpha (outside [1,2]): generic slow path
        return _entmax_bisect_numpy(x, a_eff)

    shards = x.reshape(N_CORES, ROWS_PER_CORE, K)
    in_maps = [{"x": shards[i]} for i in range(N_CORES)]

    nc = _get_nc()
    res = run_bass_kernel_spmd(nc, in_maps, core_ids=list(range(N_CORES)))
    out = np.stack([res.results[i]["out"] for i in range(N_CORES)])
    return out.reshape(B, H, Q, K)


# revision 27
# speedup vs baseline: 1.1009x; 1.0331x over previous
"""Trainium2 Bass kernel for nn_AlphaEntmax (entmax-bisect over last axis).

Key math fact: the module's ClampMin/ClampMax composition maps any alpha in
[1,2] to exactly 2.0, so the reference computes sparsemax (alpha=2) per row:
    p = relu(x - tau) / sum(relu(x - tau)),  tau s.t. sum(relu(x - tau)) = 1
We solve for tau with 4 over-relaxed Newton steps from tau0 = rowmax - 0.8
(tau' = tau + lam*(r-1)/c with r = sum(relu(x-tau)), c = count(x > tau),
lam = [1.15, 1.10, 1.05, 1.0]), then emit p = relu(x - tau) from the fp16
shadow (sum==1 at convergence; the reference's own normalize brings both
within tolerance; fp16 sourcing costs ~2e-3 of the 2e-2 budget).

Architecture: OWNERSHIP LANES — each engine owns whole tiles end-to-end so
no per-slot cross-engine barrier exists (GPSIMD cannot run compute: the
neuronxcc ISA check rejects TensorScalarPtr on Pool).
  - VectorE-owned tiles (48): per slot a min-pass  acc = sum(min(x,tau))
    (tensor_scalar op0=min accum=add, fp16 4x mode ~300ns) and an is_gt
    count pass. r = Sx - acc with Sx = sum(x) computed once per tile: the
    min-sum's magnitude is ~|Sx| ~ 30, so the HW f32 accumulator drift is
    ~1e-5. (The cheaper-looking dual  sum(max(x,tau)) = K*tau + r  has
    accumulated magnitude ~2500 and drifts ~5e-2 on HW — unusable.)
  - ScalarE-owned tiles (16): ACT Relu(x + ntau, accum) gives r directly
    (relu terms are tiny, no drift) and ACT Sign(x + ntau, accum) gives
    K - 2c; iterates in ntau space. ACT is dtype-independent so it reads
    the fp16 shadow; ~1.1us per pass.
  - the fp16 (not bf16) shadow keeps the quantization floor ~8x lower at
    the same 4x DVE speed. All update chains are small [128,4] VectorE ops.
  - finals: owner engine emits relu(x_h - tau) to an f32 buffer; stores
    ride the Pool SWDGE queue, loads the SP queue (no head-of-line mixing).
  - emission: every lane unit gets a virtual-time stamp (its serial cost
    on the owning engine); units from all lanes are merged in stamp order
    so the in-order engines always interleave across tiles; loads are
    stamped ~15us before their tile's first use.
DMA is the roofline: 32MB in + 32MB out per core over ~360GB/s ≈ 186us.

Sharding: x [8,16,512,1024] is split along the batch axis, one batch entry
(8192 rows of 1024) per NeuronCore; no cross-core communication.
"""

import numpy as np

B, H, Q, K = 8, 16, 512, 1024
N_CORES = 8
P = 128
ROWS_PER_CORE = (B // N_CORES) * H * Q  # 8192
N_TILES = ROWS_PER_CORE // P  # 64
G = 4  # tiles per group (DMA/stats granularity)
N_GROUPS = N_TILES // G  # 16
N_SLOTS = 4
D0 = 0.8  # tau0 = rowmax - D0
LAM = [1.15, 1.10, 1.05, 1.0]  # per-slot Newton over-relaxation
# group indices owned by ScalarE (spread through the run for even pacing)
S_GROUPS = (4, 9, 14)
# groups whose final runs on VectorE (the last-consumed groups: keeps the
# drain off ScalarE's tail); all other finals run on ScalarE
V_FINAL_GROUPS = (13, 15)
PREFETCH_NS = 16000.0  # how far ahead (virtual time) loads are stamped
BUFS = {"xp": 6, "hp": 7, "op": 3, "st": 8}

# virtual-time unit costs (ns) — shape emission order only
VT = {
    "cast_v": 3900.0,   # 4 casts + 4 Sx + tau0
    "cast_s": 2600.0,   # 4 casts + tau0
    "slot_v": 2800.0,   # 4 mins + 4 counts + update
    "slot_s": 10300.0,  # 4 ACT relu + 4 ACT sign + V update
    "final_v": 2700.0,  # 4 relu finals + store issue
    "final_s": 4600.0,
}

_NC_CACHE = None


def _build_nc():
    import concourse.bacc as bacc
    import concourse.mybir as mybir
    from concourse.tile import TileContext

    f32 = mybir.dt.float32
    f16 = mybir.dt.float16
    Alu = mybir.AluOpType
    Act = mybir.ActivationFunctionType

    nc = bacc.Bacc(
        "TRN2", target_bir_lowering=False, debug=False, num_devices=N_CORES
    )
    x_ext = nc.dram_tensor("x", [ROWS_PER_CORE, K], f32, kind="ExternalInput")
    out_ext = nc.dram_tensor("out", [ROWS_PER_CORE, K], f32, kind="ExternalOutput")

    ST_NAMES = ("mx", "tau", "ntau", "sx", "acc", "cacc", "r", "c", "rcp", "stp")
    NST = len(ST_NAMES)

    with TileContext(nc) as tc:
        with (
            tc.tile_pool(name="xp", bufs=BUFS["xp"]) as xp,
            tc.tile_pool(name="hp", bufs=BUFS["hp"]) as hp,
            tc.tile_pool(name="op", bufs=BUFS["op"]) as op,
            tc.tile_pool(name="scr", bufs=1) as scr,
            tc.tile_pool(name="st", bufs=BUFS["st"]) as st,
        ):
            # engine-dedicated elementwise-output scratch (never read back)
            scrR = scr.tile([P, K], f16, tag="scrR")  # V min/Sx out
            scrC = scr.tile([P, K], f16, tag="scrC")  # V count out
            scrS = scr.tile([P, K], f16, tag="scrS", name="scrS")  # S out

            # warm the ACT table (a set containing Sign AND Relu) so the
            # one-time ~2.7us table load overlaps the first DMA
            nc.scalar.activation(
                scrS[:, :1], nc.const_aps.aps[(f32, 0.0)], Act.Sign
            )
            nc.scalar.activation(
                scrS[:, 1:2], nc.const_aps.aps[(f32, 0.0)], Act.Relu
            )

            xbs, xhs, stts = {}, {}, {}

            def emit_load(g):
                rows = slice(g * G * P, (g + 1) * G * P)
                x_dram = x_ext.ap()[rows, :].rearrange("(t p) k -> p t k", p=P)
                xb = xp.tile([P, G * K], f32, tag="xb")
                if g == 0:
                    # per-tile loads so the first cast starts ~4x sooner
                    for j in range(G):
                        nc.sync.dma_start(
                            out=xb[:, j * K : (j + 1) * K], in_=x_dram[:, j, :]
                        )
                else:
                    nc.sync.dma_start(
                        out=xb[:].rearrange("p (t k) -> p t k", t=G), in_=x_dram
                    )
                xbs[g] = xb

            def emit_cast(g):
                """fp16 shadow + row max; V-owned also get Sx; then the
                slot-0 iterate (tau for V-owned, ntau for S-owned)."""
                on_s = g in S_GROUPS
                xb = xbs.pop(g)
                xh = hp.tile([P, G * K], f16, tag="xh")
                xhs[g] = xh
                st_t = st.tile([P, NST * G], f32, tag="st")
                stt = {
                    n: st_t[:, k * G : (k + 1) * G]
                    for k, n in enumerate(ST_NAMES)
                }
                stts[g] = stt
                for j in range(G):
                    sl1 = slice(j, j + 1)
                    nc.vector.tensor_scalar(
                        xh[:, j * K : (j + 1) * K], xb[:, j * K : (j + 1) * K],
                        0.0, None, Alu.add, Alu.max,
                        accum_out=stt["mx"][:, sl1],
                    )
                    if not on_s:
                        nc.vector.tensor_scalar(
                            scrR[:], xh[:, j * K : (j + 1) * K],
                            0.0, None, Alu.add, Alu.add,
                            accum_out=stt["sx"][:, sl1],
                        )
                # tau0 = mx - D0
                nc.vector.tensor_scalar(
                    stt["tau"][:], stt["mx"][:], -D0, None, Alu.add
                )
                if on_s:
                    # ScalarE ACT biases need ntau = -tau
                    nc.vector.tensor_scalar(
                        stt["ntau"][:], stt["tau"][:], -1.0, None, Alu.mult
                    )

            def emit_slot_v(g, i):
                stt = stts[g]
                xh = xhs[g]
                tau = stt["tau"]
                for j in range(G):
                    sl1 = slice(j, j + 1)
                    xhj = xh[:, j * K : (j + 1) * K]
                    # acc = sum(min(x, tau));  r = Sx - acc
                    nc.vector.tensor_scalar(
                        scrR[:], xhj, tau[:, sl1], None, Alu.min, Alu.add,
                        accum_out=stt["acc"][:, sl1],
                    )
                    nc.vector.tensor_scalar(
                        scrC[:], xhj, tau[:, sl1], None, Alu.is_gt, Alu.add,
                        accum_out=stt["cacc"][:, sl1],
                    )
                # ---- update chain ----
                nc.vector.tensor_tensor(
                    stt["r"][:], stt["sx"][:], stt["acc"][:], Alu.subtract
                )
                nc.vector.tensor_scalar_max(stt["c"][:], stt["cacc"][:], 1.0)
                nc.vector.reciprocal(stt["rcp"][:], stt["c"][:])
                nc.vector.scalar_tensor_tensor(
                    stt["stp"][:], stt["r"][:], -1.0, stt["rcp"][:],
                    Alu.add, Alu.mult,
                )
                nc.vector.scalar_tensor_tensor(
                    tau[:], stt["stp"][:], float(LAM[i]), tau[:],
                    Alu.mult, Alu.add,
                )

            def emit_slot_s(g, i):
                stt = stts[g]
                xh = xhs[g]
                ntau = stt["ntau"]
                for j in range(G):
                    sl1 = slice(j, j + 1)
                    xhj = xh[:, j * K : (j + 1) * K]
                    # r = sum(relu(x + ntau))  — tiny terms, drift-free
                    nc.scalar.activation(
                        scrS[:], xhj, Act.Relu, bias=ntau[:, sl1],
                        accum_out=stt["r"][:, sl1],
                    )
                    # cacc = sum(sign(x + ntau)) = 2c - K
                    nc.scalar.activation(
                        scrS[:], xhj, Act.Sign, bias=ntau[:, sl1],
                        accum_out=stt["cacc"][:, sl1],
                    )
                # ---- update chain (VectorE) ----
                # c = (cacc + K)/2, guard >= 1
                nc.vector.tensor_scalar(
                    stt["c"][:], stt["cacc"][:], 0.5, float(K) * 0.5,
                    Alu.mult, Alu.add,
                )
                nc.vector.tensor_scalar_max(stt["c"][:], stt["c"][:], 1.0)
                nc.vector.reciprocal(stt["rcp"][:], stt["c"][:])
                nc.vector.scalar_tensor_tensor(
                    stt["stp"][:], stt["r"][:], -1.0, stt["rcp"][:],
                    Alu.add, Alu.mult,
                )
                # tau += lam * stp;  ntau = -tau
                nc.vector.scalar_tensor_tensor(
                    stt["tau"][:], stt["stp"][:], float(LAM[i]), stt["tau"][:],
                    Alu.mult, Alu.add,
                )
                nc.vector.tensor_scalar(
                    ntau[:], stt["tau"][:], -1.0, None, Alu.mult
                )

            def emit_final(g):
                final_v = g in V_FINAL_GROUPS
                stt = stts.pop(g)
                xh = xhs.pop(g)
                rows = slice(g * G * P, (g + 1) * G * P)
                o_dram = out_ext.ap()[rows, :].rearrange("(t p) k -> p t k", p=P)
                ob = op.tile([P, G * K], f32, tag="ob")
                if not final_v and g not in S_GROUPS:
                    # V-owned group with ScalarE final: materialize ntau once
                    nc.vector.tensor_scalar(
                        stt["ntau"][:], stt["tau"][:], -1.0, None, Alu.mult
                    )
                for j in range(G):
                    sl1 = slice(j, j + 1)
                    xhj = xh[:, j * K : (j + 1) * K]
                    obj = ob[:, j * K : (j + 1) * K]
                    if final_v:
                        nc.vector.tensor_scalar(
                            obj, xhj, stt["tau"][:, sl1], 0.0,
                            Alu.subtract, Alu.max,
                        )
                    else:
                        nc.scalar.activation(
                            obj, xhj, Act.Relu, bias=stt["ntau"][:, sl1]
                        )
                nc.gpsimd.dma_start(
                    out=o_dram, in_=ob[:].rearrange("p (t k) -> p t k", t=G)
                )

            # ---- build the two lanes as (vtime, seq, emit_fn) events ----
            events = []
            seq = 0

            def push(t, fn):
                nonlocal seq
                events.append((t, seq, fn))
                seq += 1

            v_groups = [g for g in range(N_GROUPS) if g not in S_GROUPS]
            s_groups = [g for g in range(N_GROUPS) if g in S_GROUPS]

            cast_time = {}
            vt = 0.0
            for g in v_groups:
                cast_time[g] = vt
                push(vt, lambda g=g: emit_cast(g))
                vt += VT["cast_v"]
                for i in range(N_SLOTS):
                    push(vt, lambda g=g, i=i: emit_slot_v(g, i))
                    vt += VT["slot_v"]
                push(vt, lambda g=g: emit_final(g))
                vt += VT["final_v"]

            vt = 0.0
            for g in s_groups:
                cast_time[g] = vt
                # the S-lane's cast runs on V; stamp it just before use
                push(vt, lambda g=g: emit_cast(g))
                vt += VT["cast_s"]
                for i in range(N_SLOTS):
                    push(vt, lambda g=g, i=i: emit_slot_s(g, i))
                    vt += VT["slot_s"]
                push(vt, lambda g=g: emit_final(g))
                vt += VT["final_s"]

            for g in range(N_GROUPS):
                push(max(cast_time[g] - PREFETCH_NS, -1.0 - (N_GROUPS - g)),
                     lambda g=g: emit_load(g))

            events.sort(key=lambda e: (e[0], e[1]))
            for _, _, fn in events:
                fn()

    nc.compile()
    return nc


def _get_nc():
    global _NC_CACHE
    if _NC_CACHE is None:
        _NC_CACHE = _build_nc()
    return _NC_CACHE


def _effective_alpha(alpha):
    # the module's ClampMin/ClampMax pair, verbatim in numpy
    a = np.asarray(alpha, dtype=np.float32)
    a = np.maximum(np.minimum(a, 0.0) - 1.0, 0.0) + 1.0 + np.maximum(a, 0.0)
    a = np.minimum(np.maximum(a, 0.0) - 2.0, 0.0) + 2.0 + np.minimum(a, 0.0)
    return a.astype(np.float32)


def _entmax_bisect_numpy(x, a, n_iter=50):
    """Generic-alpha fallback replicating the reference bisection in f32.
    Never taken for alpha in [1,2] (the clamp maps those to exactly 2.0)."""
    f32 = np.float32
    X = x.reshape(-1, K).astype(np.float32)
    am1 = (np.broadcast_to(a.reshape(1, H), (B, H)).reshape(-1)[
        np.arange(X.shape[0]) // Q
    ].astype(np.float32) - f32(1.0))[:, None]
    Xs = (X * am1).astype(np.float32)

    def p(s):
        pos = s > 0
        return np.where(
            pos, np.power(np.where(pos, s, f32(1.0)), (f32(1.0) / am1)), f32(0.0)
        ).astype(np.float32)

    mx = Xs.max(axis=1, keepdims=True).astype(np.float32)
    tau_lo = (mx - f32(1.0)).astype(np.float32)
    tau_hi = (mx - np.power(f32(1.0 / K), am1)).astype(np.float32)
    f_lo = (p(Xs - tau_lo).sum(axis=1, dtype=np.float32)[:, None] - f32(1.0)).astype(
        np.float32
    )
    dm = (tau_hi - tau_lo).astype(np.float32)
    tau_m = tau_lo.copy()
    for _ in range(n_iter):
        dm = (dm * f32(0.5)).astype(np.float32)
        tau_m = (tau_lo + dm).astype(np.float32)
        f_m = (p(Xs - tau_m).sum(axis=1, dtype=np.float32)[:, None] - f32(1.0)).astype(
            np.float32
        )
        tau_lo = np.where(f_m * f_lo >= 0, tau_m, tau_lo).astype(np.float32)
    pm = p(Xs - tau_m)
    s = pm.sum(axis=1, dtype=np.float32).astype(np.float32)[:, None]
    return (pm / s).astype(np.float32).reshape(B, H, Q, K)


def kernel(**inputs) -> np.ndarray:
    from concourse.bass_utils import run_bass_kernel_spmd

    x = np.ascontiguousarray(np.asarray(inputs["x"], dtype=np.float32))
    alpha = np.asarray(inputs.get("alpha", np.full((1, H), 1.5, np.float32)))
    a_eff = _effective_alpha(alpha)
    if not np.all(a_eff == np.float32(2.0)):
        # out-of-distribution alpha (outside [1,2]): generic slow path
        return _entmax_bisect_numpy(x, a_eff)

    shards = x.reshape(N_CORES, ROWS_PER_CORE, K)
    in_maps = [{"x": shards[i]} for i in range(N_CORES)]

    nc = _get_nc()
    res = run_bass_kernel_spmd(nc, in_maps, core_ids=list(range(N_CORES)))
    out = np.stack([res.results[i]["out"] for i in range(N_CORES)])
    return out.reshape(B, H, Q, K)


# revision 30
# speedup vs baseline: 1.1152x; 1.0130x over previous
"""Trainium2 Bass kernel for nn_AlphaEntmax (entmax-bisect over last axis).

Key math fact: the module's ClampMin/ClampMax composition maps any alpha in
[1,2] to exactly 2.0, so the reference computes sparsemax (alpha=2) per row:
    p = relu(x - tau) / sum(relu(x - tau)),  tau s.t. sum(relu(x - tau)) = 1
We solve for tau with 4 over-relaxed Newton steps from tau0 = rowmax - 0.8
(tau' = tau + lam*(r-1)/c with r = sum(relu(x-tau)), c = count(x > tau),
lam = [1.15, 1.10, 1.05, 1.0]), then emit p = relu(x - tau) from the fp16
shadow (sum==1 at convergence; the reference's own normalize brings both
within tolerance; fp16 sourcing costs ~2e-3 of the 2e-2 budget).

Architecture: OWNERSHIP LANES — each engine owns whole tiles end-to-end so
no per-slot cross-engine barrier exists (GPSIMD cannot run compute: the
neuronxcc ISA check rejects TensorScalarPtr on Pool).
  - VectorE-owned tiles (48): per slot a min-pass  acc = sum(min(x,tau))
    (tensor_scalar op0=min accum=add, fp16 4x mode ~300ns) and an is_gt
    count pass. r = Sx - acc with Sx = sum(x) computed once per tile: the
    min-sum's magnitude is ~|Sx| ~ 30, so the HW f32 accumulator drift is
    ~1e-5. (The cheaper-looking dual  sum(max(x,tau)) = K*tau + r  has
    accumulated magnitude ~2500 and drifts ~5e-2 on HW — unusable.)
  - ScalarE-owned tiles (16): ACT Relu(x + ntau, accum) gives r directly
    (relu terms are tiny, no drift) and ACT Sign(x + ntau, accum) gives
    K - 2c; iterates in ntau space. ACT is dtype-independent so it reads
    the fp16 shadow; ~1.1us per pass.
  - the fp16 (not bf16) shadow keeps the quantization floor ~8x lower at
    the same 4x DVE speed. All update chains are small [128,4] VectorE ops.
  - finals: owner engine emits relu(x_h - tau) to an f32 buffer; stores
    ride the Pool SWDGE queue, loads the SP queue (no head-of-line mixing).
  - emission: every lane unit gets a virtual-time stamp (its serial cost
    on the owning engine); units from all lanes are merged in stamp order
    so the in-order engines always interleave across tiles; loads are
    stamped ~15us before their tile's first use.
DMA is the roofline: 32MB in + 32MB out per core over ~360GB/s ≈ 186us.

Sharding: x [8,16,512,1024] is split along the batch axis, one batch entry
(8192 rows of 1024) per NeuronCore; no cross-core communication.
"""

import numpy as np

B, H, Q, K = 8, 16, 512, 1024
N_CORES = 8
P = 128
ROWS_PER_CORE = (B // N_CORES) * H * Q  # 8192
N_TILES = ROWS_PER_CORE // P  # 64
G = 4  # tiles per group (DMA/stats granularity)
N_GROUPS = N_TILES // G  # 16
N_SLOTS = 4
D0 = 0.8  # tau0 = rowmax - D0
LAM = [1.15, 1.10, 1.05, 1.0]  # per-slot Newton over-relaxation
# group indices owned by ScalarE (spread through the run for even pacing)
S_GROUPS = (4, 9, 14)
# groups whose final runs on VectorE (the last-consumed groups: keeps the
# drain off ScalarE's tail); all other finals run on ScalarE
V_FINAL_GROUPS = (13, 15)
PREFETCH_NS = 13000.0  # how far ahead (virtual time) loads are stamped
BUFS = {"xp": 5, "hp": 7, "op": 3, "st": 8}

# virtual-time unit costs (ns) — shape emission order only
VT = {
    "cast_v": 3900.0,   # 4 casts + 4 Sx + tau0
    "cast_s": 2600.0,   # 4 casts + tau0
    "slot_v": 2800.0,   # 4 mins + 4 counts + update
    "slot_s": 10300.0,  # 4 ACT relu + 4 ACT sign + V update
    "final_v": 2700.0,  # 4 relu finals + store issue
    "final_s": 4600.0,
}

_NC_CACHE = None


def _build_nc():
    import concourse.bacc as bacc
    import concourse.mybir as mybir
    from concourse.tile import TileContext

    f32 = mybir.dt.float32
    f16 = mybir.dt.float16
    Alu = mybir.AluOpType
    Act = mybir.ActivationFunctionType

    nc = bacc.Bacc(
        "TRN2", target_bir_lowering=False, debug=False, num_devices=N_CORES
    )
    x_ext = nc.dram_tensor("x", [ROWS_PER_CORE, K], f32, kind="ExternalInput")
    out_ext = nc.dram_tensor("out", [ROWS_PER_CORE, K], f32, kind="ExternalOutput")

    ST_NAMES = ("mx", "tau", "ntau", "sx", "acc", "cacc", "r", "c", "rcp", "stp")
    NST = len(ST_NAMES)

    with TileContext(nc) as tc:
        with (
            tc.tile_pool(name="xp", bufs=BUFS["xp"]) as xp,
            tc.tile_pool(name="hp", bufs=BUFS["hp"]) as hp,
            tc.tile_pool(name="op", bufs=BUFS["op"]) as op,
            tc.tile_pool(name="scr", bufs=1) as scr,
            tc.tile_pool(name="st", bufs=BUFS["st"]) as st,
        ):
            # engine-dedicated elementwise-output scratch (never read back)
            scrR = scr.tile([P, K], f16, tag="scrR")  # V min/Sx out
            scrC = scr.tile([P, K], f16, tag="scrC")  # V count out
            scrS = scr.tile([P, K], f16, tag="scrS", name="scrS")  # S out

            # warm the ACT table (a set containing Sign AND Relu) so the
            # one-time ~2.7us table load overlaps the first DMA
            nc.scalar.activation(
                scrS[:, :1], nc.const_aps.aps[(f32, 0.0)], Act.Sign
            )
            nc.scalar.activation(
                scrS[:, 1:2], nc.const_aps.aps[(f32, 0.0)], Act.Relu
            )

            xbs, xhs, stts = {}, {}, {}
            chunk_of = {}  # group -> (chunk_key, position)
            chunk_stt = {}  # chunk_key -> stats dict over [P, 10*ntiles]

            def emit_load(g):
                rows = slice(g * G * P, (g + 1) * G * P)
                x_dram = x_ext.ap()[rows, :].rearrange("(t p) k -> p t k", p=P)
                xb = xp.tile([P, G * K], f32, tag="xb")
                if g == 0:
                    # per-tile loads so the first cast starts ~4x sooner
                    for j in range(G):
                        nc.sync.dma_start(
                            out=xb[:, j * K : (j + 1) * K], in_=x_dram[:, j, :]
                        )
                else:
                    nc.sync.dma_start(
                        out=xb[:].rearrange("p (t k) -> p t k", t=G), in_=x_dram
                    )
                xbs[g] = xb

            def emit_cast_chunk(chunk):
                """fp16 shadow + row max for every tile of the chunk; V-owned
                chunks also get Sx; one tau0 op over the whole stats width."""
                on_s = chunk[0] in S_GROUPS
                ncols = G * len(chunk)
                st_t = st.tile([P, NST * ncols], f32, tag="st")
                stt = {
                    n: st_t[:, k * ncols : (k + 1) * ncols]
                    for k, n in enumerate(ST_NAMES)
                }
                chunk_stt[chunk[0]] = stt
                for ci, g in enumerate(chunk):
                    chunk_of[g] = (chunk[0], ci)
                    xb = xbs.pop(g)
                    xh = hp.tile([P, G * K], f16, tag="xh")
                    xhs[g] = xh
                    for j in range(G):
                        sl1 = slice(ci * G + j, ci * G + j + 1)
                        nc.vector.tensor_scalar(
                            xh[:, j * K : (j + 1) * K],
                            xb[:, j * K : (j + 1) * K],
                            0.0, None, Alu.add, Alu.max,
                            accum_out=stt["mx"][:, sl1],
                        )
                        if not on_s:
                            nc.vector.tensor_scalar(
                                scrR[:], xh[:, j * K : (j + 1) * K],
                                0.0, None, Alu.add, Alu.add,
                                accum_out=stt["sx"][:, sl1],
                            )
                # tau0 = mx - D0
                nc.vector.tensor_scalar(
                    stt["tau"][:], stt["mx"][:], -D0, None, Alu.add
                )
                if on_s:
                    # ScalarE ACT biases need ntau = -tau
                    nc.vector.tensor_scalar(
                        stt["ntau"][:], stt["tau"][:], -1.0, None, Alu.mult
                    )

            def emit_slot_v_chunk(chunk, i):
                stt = chunk_stt[chunk[0]]
                tau = stt["tau"]
                for ci, g in enumerate(chunk):
                    xh = xhs[g]
                    for j in range(G):
                        sl1 = slice(ci * G + j, ci * G + j + 1)
                        xhj = xh[:, j * K : (j + 1) * K]
                        # acc = sum(min(x, tau));  r = Sx - acc
                        nc.vector.tensor_scalar(
                            scrR[:], xhj, tau[:, sl1], None, Alu.min, Alu.add,
                            accum_out=stt["acc"][:, sl1],
                        )
                        nc.vector.tensor_scalar(
                            scrC[:], xhj, tau[:, sl1], None, Alu.is_gt, Alu.add,
                            accum_out=stt["cacc"][:, sl1],
                        )
                # ---- one update chain for the whole chunk ----
                nc.vector.tensor_tensor(
                    stt["r"][:], stt["sx"][:], stt["acc"][:], Alu.subtract
                )
                nc.vector.tensor_scalar_max(stt["c"][:], stt["cacc"][:], 1.0)
                nc.vector.reciprocal(stt["rcp"][:], stt["c"][:])
                nc.vector.scalar_tensor_tensor(
                    stt["stp"][:], stt["r"][:], -1.0, stt["rcp"][:],
                    Alu.add, Alu.mult,
                )
                nc.vector.scalar_tensor_tensor(
                    tau[:], stt["stp"][:], float(LAM[i]), tau[:],
                    Alu.mult, Alu.add,
                )

            def emit_slot_s(g, i):
                stt = chunk_stt[g]
                xh = xhs[g]
                ntau = stt["ntau"]
                for j in range(G):
                    sl1 = slice(j, j + 1)
                    xhj = xh[:, j * K : (j + 1) * K]
                    # r = sum(relu(x + ntau))  — tiny terms, drift-free
                    nc.scalar.activation(
                        scrS[:], xhj, Act.Relu, bias=ntau[:, sl1],
                        accum_out=stt["r"][:, sl1],
                    )
                    # cacc = sum(sign(x + ntau)) = 2c - K
                    nc.scalar.activation(
                        scrS[:], xhj, Act.Sign, bias=ntau[:, sl1],
                        accum_out=stt["cacc"][:, sl1],
                    )
                # ---- update chain (VectorE) ----
                # c = (cacc + K)/2, guard >= 1
                nc.vector.tensor_scalar(
                    stt["c"][:], stt["cacc"][:], 0.5, float(K) * 0.5,
                    Alu.mult, Alu.add,
                )
                nc.vector.tensor_scalar_max(stt["c"][:], stt["c"][:], 1.0)
                nc.vector.reciprocal(stt["rcp"][:], stt["c"][:])
                nc.vector.scalar_tensor_tensor(
                    stt["stp"][:], stt["r"][:], -1.0, stt["rcp"][:],
                    Alu.add, Alu.mult,
                )
                # tau += lam * stp;  ntau = -tau
                nc.vector.scalar_tensor_tensor(
                    stt["tau"][:], stt["stp"][:], float(LAM[i]), stt["tau"][:],
                    Alu.mult, Alu.add,
                )
                nc.vector.tensor_scalar(
                    ntau[:], stt["tau"][:], -1.0, None, Alu.mult
                )

            def emit_final(g):
                final_v = g in V_FINAL_GROUPS
                ckey, ci = chunk_of[g]
                stt = chunk_stt[ckey]
                xh = xhs.pop(g)
                rows = slice(g * G * P, (g + 1) * G * P)
                o_dram = out_ext.ap()[rows, :].rearrange("(t p) k -> p t k", p=P)
                ob = op.tile([P, G * K], f32, tag="ob")
                if not final_v and g not in S_GROUPS:
                    # ScalarE final for a V-owned group: materialize ntau
                    sl = slice(ci * G, (ci + 1) * G)
                    nc.vector.tensor_scalar(
                        stt["ntau"][:, sl], stt["tau"][:, sl], -1.0, None,
                        Alu.mult,
                    )
                for j in range(G):
                    sl1 = slice(ci * G + j, ci * G + j + 1)
                    xhj = xh[:, j * K : (j + 1) * K]
                    obj = ob[:, j * K : (j + 1) * K]
                    if final_v:
                        nc.vector.tensor_scalar(
                            obj, xhj, stt["tau"][:, sl1], 0.0,
                            Alu.subtract, Alu.max,
                        )
                    else:
                        nc.scalar.activation(
                            obj, xhj, Act.Relu, bias=stt["ntau"][:, sl1]
                        )
                nc.gpsimd.dma_start(
                    out=o_dram, in_=ob[:].rearrange("p (t k) -> p t k", t=G)
                )

            # ---- build the two lanes as (vtime, seq, emit_fn) events ----
            events = []
            seq = 0

            def push(t, fn):
                nonlocal seq
                events.append((t, seq, fn))
                seq += 1

            v_list = [g for g in range(N_GROUPS) if g not in S_GROUPS]
            s_list = [g for g in range(N_GROUPS) if g in S_GROUPS]
            v_chunks = [v_list[i : i + 2] for i in range(0, len(v_list), 2)]

            cast_time = {}
            vt = 0.0
            for chunk in v_chunks:
                for g in chunk:
                    cast_time[g] = vt
                push(vt, lambda c=chunk: emit_cast_chunk(c))
                vt += VT["cast_v"] * len(chunk)
                for i in range(N_SLOTS):
                    push(vt, lambda c=chunk, i=i: emit_slot_v_chunk(c, i))
                    vt += VT["slot_v"] * len(chunk)
                for g in chunk:
                    push(vt, lambda g=g: emit_final(g))
                    vt += VT["final_v"]
            v_span = vt

            # stretch the ScalarE lane across the whole run so its own
            # groups interleave evenly with the finals it also executes
            s_span = len(s_list) * (
                VT["cast_s"] + N_SLOTS * VT["slot_s"] + VT["final_s"]
            )
            stretch = max(v_span / s_span, 1.0) if s_span else 1.0
            vt = 0.0
            for g in s_list:
                cast_time[g] = vt * stretch
                push(vt * stretch, lambda g=g: emit_cast_chunk([g]))
                vt += VT["cast_s"]
                for i in range(N_SLOTS):
                    push(vt * stretch, lambda g=g, i=i: emit_slot_s(g, i))
                    vt += VT["slot_s"]
                push(vt * stretch, lambda g=g: emit_final(g))
                vt += VT["final_s"]

            for g in range(N_GROUPS):
                push(max(cast_time[g] - PREFETCH_NS, -1.0 - (N_GROUPS - g)),
                     lambda g=g: emit_load(g))

            events.sort(key=lambda e: (e[0], e[1]))
            for _, _, fn in events:
                fn()

    nc.compile()
    return nc


def _get_nc():
    global _NC_CACHE
    if _NC_CACHE is None:
        _NC_CACHE = _build_nc()
    return _NC_CACHE


def _effective_alpha(alpha):
    # the module's ClampMin/ClampMax pair, verbatim in numpy
    a = np.asarray(alpha, dtype=np.float32)
    a = np.maximum(np.minimum(a, 0.0) - 1.0, 0.0) + 1.0 + np.maximum(a, 0.0)
    a = np.minimum(np.maximum(a, 0.0) - 2.0, 0.0) + 2.0 + np.minimum(a, 0.0)
    return a.astype(np.float32)


def _entmax_bisect_numpy(x, a, n_iter=50):
    """Generic-alpha fallback replicating the reference bisection in f32.
    Never taken for alpha in [1,2] (the clamp maps those to exactly 2.0)."""
    f32 = np.float32
    X = x.reshape(-1, K).astype(np.float32)
    am1 = (np.broadcast_to(a.reshape(1, H), (B, H)).reshape(-1)[
        np.arange(X.shape[0]) // Q
    ].astype(np.float32) - f32(1.0))[:, None]
    Xs = (X * am1).astype(np.float32)

    def p(s):
        pos = s > 0
        return np.where(
            pos, np.power(np.where(pos, s, f32(1.0)), (f32(1.0) / am1)), f32(0.0)
        ).astype(np.float32)

    mx = Xs.max(axis=1, keepdims=True).astype(np.float32)
    tau_lo = (mx - f32(1.0)).astype(np.float32)
    tau_hi = (mx - np.power(f32(1.0 / K), am1)).astype(np.float32)
    f_lo = (p(Xs - tau_lo).sum(axis=1, dtype=np.float32)[:, None] - f32(1.0)).astype(
        np.float32
    )
    dm = (tau_hi - tau_lo).astype(np.float32)
    tau_m = tau_lo.copy()
    for _ in range(n_iter):
        dm = (dm * f32(0.5)).astype(np.float32)
        tau_m = (tau_lo + dm).astype(np.float32)
        f_m = (p(Xs - tau_m).sum(axis=1, dtype=np.float32)[:, None] - f32(1.0)).astype(
            np.float32
        )
        tau_lo = np.where(f_m * f_lo >= 0, tau_m, tau_lo).astype(np.float32)
    pm = p(Xs - tau_m)
    s = pm.sum(axis=1, dtype=np.float32).astype(np.float32)[:, None]
    return (pm / s).astype(np.float32).reshape(B, H, Q, K)


def kernel(**inputs) -> np.ndarray:
    from concourse.bass_utils import run_bass_kernel_spmd

    x = np.ascontiguousarray(np.asarray(inputs["x"], dtype=np.float32))
    alpha = np.asarray(inputs.get("alpha", np.full((1, H), 1.5, np.float32)))
    a_eff = _effective_alpha(alpha)
    if not np.all(a_eff == np.float32(2.0)):
        # out-of-distribution alpha (outside [1,2]): generic slow path
        return _entmax_bisect_numpy(x, a_eff)

    shards = x.reshape(N_CORES, ROWS_PER_CORE, K)
    in_maps = [{"x": shards[i]} for i in range(N_CORES)]

    nc = _get_nc()
    res = run_bass_kernel_spmd(nc, in_maps, core_ids=list(range(N_CORES)))
    out = np.stack([res.results[i]["out"] for i in range(N_CORES)])
    return out.reshape(B, H, Q, K)


# revision 31
# speedup vs baseline: 1.1227x; 1.0067x over previous
"""Trainium2 Bass kernel for nn_AlphaEntmax (entmax-bisect over last axis).

Key math fact: the module's ClampMin/ClampMax composition maps any alpha in
[1,2] to exactly 2.0, so the reference computes sparsemax (alpha=2) per row:
    p = relu(x - tau) / sum(relu(x - tau)),  tau s.t. sum(relu(x - tau)) = 1
We solve for tau with 4 over-relaxed Newton steps from tau0 = rowmax - 0.8
(tau' = tau + lam*(r-1)/c with r = sum(relu(x-tau)), c = count(x > tau),
lam = [1.15, 1.10, 1.05, 1.0]), then emit p = relu(x - tau) from the fp16
shadow (sum==1 at convergence; the reference's own normalize brings both
within tolerance; fp16 sourcing costs ~2e-3 of the 2e-2 budget).

Architecture: OWNERSHIP LANES — each engine owns whole tiles end-to-end so
no per-slot cross-engine barrier exists (GPSIMD cannot run compute: the
neuronxcc ISA check rejects TensorScalarPtr on Pool).
  - VectorE-owned tiles (48): per slot a min-pass  acc = sum(min(x,tau))
    (tensor_scalar op0=min accum=add, fp16 4x mode ~300ns) and an is_gt
    count pass. r = Sx - acc with Sx = sum(x) computed once per tile: the
    min-sum's magnitude is ~|Sx| ~ 30, so the HW f32 accumulator drift is
    ~1e-5. (The cheaper-looking dual  sum(max(x,tau)) = K*tau + r  has
    accumulated magnitude ~2500 and drifts ~5e-2 on HW — unusable.)
  - ScalarE-owned tiles (16): ACT Relu(x + ntau, accum) gives r directly
    (relu terms are tiny, no drift) and ACT Sign(x + ntau, accum) gives
    K - 2c; iterates in ntau space. ACT is dtype-independent so it reads
    the fp16 shadow; ~1.1us per pass.
  - the fp16 (not bf16) shadow keeps the quantization floor ~8x lower at
    the same 4x DVE speed. All update chains are small [128,4] VectorE ops.
  - finals: owner engine emits relu(x_h - tau) to an f32 buffer; stores
    ride the Pool SWDGE queue, loads the SP queue (no head-of-line mixing).
  - emission: every lane unit gets a virtual-time stamp (its serial cost
    on the owning engine); units from all lanes are merged in stamp order
    so the in-order engines always interleave across tiles; loads are
    stamped ~15us before their tile's first use.
DMA is the roofline: 32MB in + 32MB out per core over ~360GB/s ≈ 186us.

Sharding: x [8,16,512,1024] is split along the batch axis, one batch entry
(8192 rows of 1024) per NeuronCore; no cross-core communication.
"""

import numpy as np

B, H, Q, K = 8, 16, 512, 1024
N_CORES = 8
P = 128
ROWS_PER_CORE = (B // N_CORES) * H * Q  # 8192
N_TILES = ROWS_PER_CORE // P  # 64
G = 4  # tiles per group (DMA/stats granularity)
N_GROUPS = N_TILES // G  # 16
N_SLOTS = 4
D0 = 0.8  # tau0 = rowmax - D0
LAM = [1.15, 1.10, 1.05, 1.0]  # per-slot Newton over-relaxation
# group indices owned by ScalarE (spread through the run for even pacing)
S_GROUPS = (4, 9, 14)
# groups whose final runs on VectorE (the last-consumed groups: keeps the
# drain off ScalarE's tail); all other finals run on ScalarE
V_FINAL_GROUPS = (13, 15)
PREFETCH_NS = 13000.0  # how far ahead (virtual time) loads are stamped
BUFS = {"xp": 5, "hp": 7, "op": 3, "st": 8}

# virtual-time unit costs (ns) — shape emission order only
VT = {
    "cast_v": 3900.0,   # 4 casts + 4 Sx + tau0
    "cast_s": 2600.0,   # 4 casts + tau0
    "slot_v": 2800.0,   # 4 mins + 4 counts + update
    "slot_s": 10300.0,  # 4 ACT relu + 4 ACT sign + V update
    "final_v": 2700.0,  # 4 relu finals + store issue
    "final_s": 4600.0,
}

_NC_CACHE = None


def _build_nc():
    import concourse.bacc as bacc
    import concourse.mybir as mybir
    from concourse.tile import TileContext

    f32 = mybir.dt.float32
    f16 = mybir.dt.float16
    Alu = mybir.AluOpType
    Act = mybir.ActivationFunctionType

    nc = bacc.Bacc(
        "TRN2", target_bir_lowering=False, debug=False, num_devices=N_CORES
    )
    x_ext = nc.dram_tensor("x", [ROWS_PER_CORE, K], f32, kind="ExternalInput")
    out_ext = nc.dram_tensor("out", [ROWS_PER_CORE, K], f32, kind="ExternalOutput")

    ST_NAMES = ("mx", "tau", "ntau", "sx", "acc", "cacc", "r", "c", "rcp", "stp")
    NST = len(ST_NAMES)

    with TileContext(nc) as tc:
        with (
            tc.tile_pool(name="xp", bufs=BUFS["xp"]) as xp,
            tc.tile_pool(name="hp", bufs=BUFS["hp"]) as hp,
            tc.tile_pool(name="op", bufs=BUFS["op"]) as op,
            tc.tile_pool(name="scr", bufs=1) as scr,
            tc.tile_pool(name="st", bufs=BUFS["st"]) as st,
        ):
            # engine-dedicated elementwise-output scratch (never read back)
            scrR = scr.tile([P, K], f16, tag="scrR")  # V min/Sx out
            scrC = scr.tile([P, K], f16, tag="scrC")  # V count out
            scrS = scr.tile([P, K], f16, tag="scrS", name="scrS")  # S out

            # warm the ACT table (a set containing Sign AND Relu) so the
            # one-time ~2.7us table load overlaps the first DMA
            nc.scalar.activation(
                scrS[:, :1], nc.const_aps.aps[(f32, 0.0)], Act.Sign
            )
            nc.scalar.activation(
                scrS[:, 1:2], nc.const_aps.aps[(f32, 0.0)], Act.Relu
            )

            xbs, xhs, stts = {}, {}, {}
            chunk_of = {}  # group -> (chunk_key, position)
            chunk_stt = {}  # chunk_key -> stats dict over [P, 10*ntiles]

            def emit_load(g):
                rows = slice(g * G * P, (g + 1) * G * P)
                x_dram = x_ext.ap()[rows, :].rearrange("(t p) k -> p t k", p=P)
                xb = xp.tile([P, G * K], f32, tag="xb")
                if g == 0:
                    # per-tile loads so the first cast starts ~4x sooner
                    for j in range(G):
                        nc.sync.dma_start(
                            out=xb[:, j * K : (j + 1) * K], in_=x_dram[:, j, :]
                        )
                else:
                    nc.sync.dma_start(
                        out=xb[:].rearrange("p (t k) -> p t k", t=G), in_=x_dram
                    )
                xbs[g] = xb

            def emit_cast_chunk(chunk):
                """fp16 shadow + row max for every tile of the chunk; V-owned
                chunks also get Sx; one tau0 op over the whole stats width."""
                on_s = chunk[0] in S_GROUPS
                ncols = G * len(chunk)
                st_t = st.tile([P, NST * ncols], f32, tag="st")
                stt = {
                    n: st_t[:, k * ncols : (k + 1) * ncols]
                    for k, n in enumerate(ST_NAMES)
                }
                chunk_stt[chunk[0]] = stt
                for ci, g in enumerate(chunk):
                    chunk_of[g] = (chunk[0], ci)
                    xb = xbs.pop(g)
                    xh = hp.tile([P, G * K], f16, tag="xh")
                    xhs[g] = xh
                    for j in range(G):
                        sl1 = slice(ci * G + j, ci * G + j + 1)
                        nc.vector.tensor_scalar(
                            xh[:, j * K : (j + 1) * K],
                            xb[:, j * K : (j + 1) * K],
                            0.0, None, Alu.add, Alu.max,
                            accum_out=stt["mx"][:, sl1],
                        )
                        if not on_s:
                            nc.vector.tensor_scalar(
                                scrR[:], xh[:, j * K : (j + 1) * K],
                                0.0, None, Alu.add, Alu.add,
                                accum_out=stt["sx"][:, sl1],
                            )
                # tau0 = mx - D0
                nc.vector.tensor_scalar(
                    stt["tau"][:], stt["mx"][:], -D0, None, Alu.add
                )
                if on_s:
                    # ScalarE ACT biases need ntau = -tau
                    nc.vector.tensor_scalar(
                        stt["ntau"][:], stt["tau"][:], -1.0, None, Alu.mult
                    )

            def emit_slot_v_chunk(chunk, i):
                stt = chunk_stt[chunk[0]]
                tau = stt["tau"]
                for ci, g in enumerate(chunk):
                    xh = xhs[g]
                    for j in range(G):
                        sl1 = slice(ci * G + j, ci * G + j + 1)
                        xhj = xh[:, j * K : (j + 1) * K]
                        # acc = sum(min(x, tau));  r = Sx - acc
                        nc.vector.tensor_scalar(
                            scrR[:], xhj, tau[:, sl1], None, Alu.min, Alu.add,
                            accum_out=stt["acc"][:, sl1],
                        )
                        nc.vector.tensor_scalar(
                            scrC[:], xhj, tau[:, sl1], None, Alu.is_gt, Alu.add,
                            accum_out=stt["cacc"][:, sl1],
                        )
                # ---- one update chain for the whole chunk ----
                nc.vector.tensor_tensor(
                    stt["r"][:], stt["sx"][:], stt["acc"][:], Alu.subtract
                )
                nc.vector.tensor_scalar_max(stt["c"][:], stt["cacc"][:], 1.0)
                nc.vector.reciprocal(stt["rcp"][:], stt["c"][:])
                nc.vector.scalar_tensor_tensor(
                    stt["stp"][:], stt["r"][:], -1.0, stt["rcp"][:],
                    Alu.add, Alu.mult,
                )
                nc.vector.scalar_tensor_tensor(
                    tau[:], stt["stp"][:], float(LAM[i]), tau[:],
                    Alu.mult, Alu.add,
                )

            def emit_slot_s(g, i):
                stt = chunk_stt[g]
                xh = xhs[g]
                ntau = stt["ntau"]
                for j in range(G):
                    sl1 = slice(j, j + 1)
                    xhj = xh[:, j * K : (j + 1) * K]
                    # r = sum(relu(x + ntau))  — tiny terms, drift-free
                    nc.scalar.activation(
                        scrS[:], xhj, Act.Relu, bias=ntau[:, sl1],
                        accum_out=stt["r"][:, sl1],
                    )
                    # cacc = sum(sign(x + ntau)) = 2c - K
                    nc.scalar.activation(
                        scrS[:], xhj, Act.Sign, bias=ntau[:, sl1],
                        accum_out=stt["cacc"][:, sl1],
                    )
                # ---- update chain (VectorE) ----
                # c = (cacc + K)/2, guard >= 1
                nc.vector.tensor_scalar(
                    stt["c"][:], stt["cacc"][:], 0.5, float(K) * 0.5,
                    Alu.mult, Alu.add,
                )
                nc.vector.tensor_scalar_max(stt["c"][:], stt["c"][:], 1.0)
                nc.vector.reciprocal(stt["rcp"][:], stt["c"][:])
                nc.vector.scalar_tensor_tensor(
                    stt["stp"][:], stt["r"][:], -1.0, stt["rcp"][:],
                    Alu.add, Alu.mult,
                )
                # tau += lam * stp;  ntau = -tau
                nc.vector.scalar_tensor_tensor(
                    stt["tau"][:], stt["stp"][:], float(LAM[i]), stt["tau"][:],
                    Alu.mult, Alu.add,
                )
                nc.vector.tensor_scalar(
                    ntau[:], stt["tau"][:], -1.0, None, Alu.mult
                )

            def emit_final(g):
                final_v = g in V_FINAL_GROUPS
                ckey, ci = chunk_of[g]
                stt = chunk_stt[ckey]
                xh = xhs.pop(g)
                rows = slice(g * G * P, (g + 1) * G * P)
                o_dram = out_ext.ap()[rows, :].rearrange("(t p) k -> p t k", p=P)
                ob = op.tile([P, G * K], f32, tag="ob")
                if not final_v and g not in S_GROUPS:
                    # ScalarE final for a V-owned group: materialize ntau
                    sl = slice(ci * G, (ci + 1) * G)
                    nc.vector.tensor_scalar(
                        stt["ntau"][:, sl], stt["tau"][:, sl], -1.0, None,
                        Alu.mult,
                    )
                for j in range(G):
                    sl1 = slice(ci * G + j, ci * G + j + 1)
                    xhj = xh[:, j * K : (j + 1) * K]
                    obj = ob[:, j * K : (j + 1) * K]
                    if final_v:
                        nc.vector.tensor_scalar(
                            obj, xhj, stt["tau"][:, sl1], 0.0,
                            Alu.subtract, Alu.max,
                        )
                    else:
                        nc.scalar.activation(
                            obj, xhj, Act.Relu, bias=stt["ntau"][:, sl1]
                        )
                nc.gpsimd.dma_start(
                    out=o_dram, in_=ob[:].rearrange("p (t k) -> p t k", t=G)
                )

            # ---- build the two lanes as (vtime, seq, emit_fn) events ----
            events = []
            seq = 0

            def push(t, fn):
                nonlocal seq
                events.append((t, seq, fn))
                seq += 1

            v_list = [g for g in range(N_GROUPS) if g not in S_GROUPS]
            s_list = [g for g in range(N_GROUPS) if g in S_GROUPS]
            v_chunks = [v_list[i : i + 2] for i in range(0, len(v_list), 2)]

            cast_time = {}
            vt = 0.0
            for chunk in v_chunks:
                for g in chunk:
                    cast_time[g] = vt
                push(vt, lambda c=chunk: emit_cast_chunk(c))
                vt += VT["cast_v"] * len(chunk)
                for i in range(N_SLOTS):
                    push(vt, lambda c=chunk, i=i: emit_slot_v_chunk(c, i))
                    vt += VT["slot_v"] * len(chunk)
                for g in chunk:
                    push(vt, lambda g=g: emit_final(g))
                    vt += VT["final_v"]
            v_span = vt

            # stretch the ScalarE lane across the whole run so its own
            # groups interleave evenly with the finals it also executes
            s_span = len(s_list) * (
                VT["cast_s"] + N_SLOTS * VT["slot_s"] + VT["final_s"]
            )
            stretch = max(0.9 * v_span / s_span, 1.0) if s_span else 1.0
            S_OFF = -2000.0  # S lane leads slightly so its first load is first
            vt = 0.0
            for g in s_list:
                cast_time[g] = vt * stretch + S_OFF
                push(vt * stretch + S_OFF, lambda g=g: emit_cast_chunk([g]))
                vt += VT["cast_s"]
                for i in range(N_SLOTS):
                    push(vt * stretch + S_OFF, lambda g=g, i=i: emit_slot_s(g, i))
                    vt += VT["slot_s"]
                push(vt * stretch + S_OFF, lambda g=g: emit_final(g))
                vt += VT["final_s"]

            for g in range(N_GROUPS):
                push(cast_time[g] - PREFETCH_NS, lambda g=g: emit_load(g))

            events.sort(key=lambda e: (e[0], e[1]))
            for _, _, fn in events:
                fn()

    nc.compile()
    return nc


def _get_nc():
    global _NC_CACHE
    if _NC_CACHE is None:
        _NC_CACHE = _build_nc()
    return _NC_CACHE


def _effective_alpha(alpha):
    # the module's ClampMin/ClampMax pair, verbatim in numpy
    a = np.asarray(alpha, dtype=np.float32)
    a = np.maximum(np.minimum(a, 0.0) - 1.0, 0.0) + 1.0 + np.maximum(a, 0.0)
    a = np.minimum(np.maximum(a, 0.0) - 2.0, 0.0) + 2.0 + np.minimum(a, 0.0)
    return a.astype(np.float32)


def _entmax_bisect_numpy(x, a, n_iter=50):
    """Generic-alpha fallback replicating the reference bisection in f32.
    Never taken for alpha in [1,2] (the clamp maps those to exactly 2.0)."""
    f32 = np.float32
    X = x.reshape(-1, K).astype(np.float32)
    am1 = (np.broadcast_to(a.reshape(1, H), (B, H)).reshape(-1)[
        np.arange(X.shape[0]) // Q
    ].astype(np.float32) - f32(1.0))[:, None]
    Xs = (X * am1).astype(np.float32)

    def p(s):
        pos = s > 0
        return np.where(
            pos, np.power(np.where(pos, s, f32(1.0)), (f32(1.0) / am1)), f32(0.0)
        ).astype(np.float32)

    mx = Xs.max(axis=1, keepdims=True).astype(np.float32)
    tau_lo = (mx - f32(1.0)).astype(np.float32)
    tau_hi = (mx - np.power(f32(1.0 / K), am1)).astype(np.float32)
    f_lo = (p(Xs - tau_lo).sum(axis=1, dtype=np.float32)[:, None] - f32(1.0)).astype(
        np.float32
    )
    dm = (tau_hi - tau_lo).astype(np.float32)
    tau_m = tau_lo.copy()
    for _ in range(n_iter):
        dm = (dm * f32(0.5)).astype(np.float32)
        tau_m = (tau_lo + dm).astype(np.float32)
        f_m = (p(Xs - tau_m).sum(axis=1, dtype=np.float32)[:, None] - f32(1.0)).astype(
            np.float32
        )
        tau_lo = np.where(f_m * f_lo >= 0, tau_m, tau_lo).astype(np.float32)
    pm = p(Xs - tau_m)
    s = pm.sum(axis=1, dtype=np.float32).astype(np.float32)[:, None]
    return (pm / s).astype(np.float32).reshape(B, H, Q, K)


def kernel(**inputs) -> np.ndarray:
    from concourse.bass_utils import run_bass_kernel_spmd

    x = np.ascontiguousarray(np.asarray(inputs["x"], dtype=np.float32))
    alpha = np.asarray(inputs.get("alpha", np.full((1, H), 1.5, np.float32)))
    a_eff = _effective_alpha(alpha)
    if not np.all(a_eff == np.float32(2.0)):
        # out-of-distribution alpha (outside [1,2]): generic slow path
        return _entmax_bisect_numpy(x, a_eff)

    shards = x.reshape(N_CORES, ROWS_PER_CORE, K)
    in_maps = [{"x": shards[i]} for i in range(N_CORES)]

    nc = _get_nc()
    res = run_bass_kernel_spmd(nc, in_maps, core_ids=list(range(N_CORES)))
    out = np.stack([res.results[i]["out"] for i in range(N_CORES)])
    return out.reshape(B, H, Q, K)


# revision 32
# speedup vs baseline: 1.1478x; 1.0223x over previous
"""Trainium2 Bass kernel for nn_AlphaEntmax (entmax-bisect over last axis).

Key math fact: the module's ClampMin/ClampMax composition maps any alpha in
[1,2] to exactly 2.0, so the reference computes sparsemax (alpha=2) per row:
    p = relu(x - tau) / sum(relu(x - tau)),  tau s.t. sum(relu(x - tau)) = 1
We solve for tau with 4 over-relaxed Newton steps from tau0 = rowmax - 0.8
(tau' = tau + lam*(r-1)/c with r = sum(relu(x-tau)), c = count(x > tau),
lam = [1.15, 1.10, 1.05, 1.0]), then emit p = relu(x - tau) from the fp16
shadow (sum==1 at convergence; the reference's own normalize brings both
within tolerance; fp16 sourcing costs ~2e-3 of the 2e-2 budget).

Architecture: OWNERSHIP LANES — each engine owns whole tiles end-to-end so
no per-slot cross-engine barrier exists (GPSIMD cannot run compute: the
neuronxcc ISA check rejects TensorScalarPtr on Pool).
  - VectorE-owned tiles (48): per slot a min-pass  acc = sum(min(x,tau))
    (tensor_scalar op0=min accum=add, fp16 4x mode ~300ns) and an is_gt
    count pass. r = Sx - acc with Sx = sum(x) computed once per tile: the
    min-sum's magnitude is ~|Sx| ~ 30, so the HW f32 accumulator drift is
    ~1e-5. (The cheaper-looking dual  sum(max(x,tau)) = K*tau + r  has
    accumulated magnitude ~2500 and drifts ~5e-2 on HW — unusable.)
  - ScalarE-owned tiles (16): ACT Relu(x + ntau, accum) gives r directly
    (relu terms are tiny, no drift) and ACT Sign(x + ntau, accum) gives
    K - 2c; iterates in ntau space. ACT is dtype-independent so it reads
    the fp16 shadow; ~1.1us per pass.
  - the fp16 (not bf16) shadow keeps the quantization floor ~8x lower at
    the same 4x DVE speed. All update chains are small [128,4] VectorE ops.
  - finals: owner engine emits relu(x_h - tau) to an f32 buffer; stores
    ride the Pool SWDGE queue, loads the SP queue (no head-of-line mixing).
  - emission: every lane unit gets a virtual-time stamp (its serial cost
    on the owning engine); units from all lanes are merged in stamp order
    so the in-order engines always interleave across tiles; loads are
    stamped ~15us before their tile's first use.
DMA is the roofline: 32MB in + 32MB out per core over ~360GB/s ≈ 186us.

Sharding: x [8,16,512,1024] is split along the batch axis, one batch entry
(8192 rows of 1024) per NeuronCore; no cross-core communication.
"""

import numpy as np

B, H, Q, K = 8, 16, 512, 1024
N_CORES = 8
P = 128
ROWS_PER_CORE = (B // N_CORES) * H * Q  # 8192
N_TILES = ROWS_PER_CORE // P  # 64
G = 4  # tiles per group (DMA/stats granularity)
N_GROUPS = N_TILES // G  # 16
N_SLOTS = 4
D0 = 0.8  # tau0 = rowmax - D0
LAM = [1.15, 1.10, 1.05, 1.0]  # per-slot Newton over-relaxation
# group indices owned by ScalarE (spread through the run for even pacing);
# processed in interleaved pairs so VectorE's count+update for one group
# overlaps ScalarE's r-passes on the other
S_GROUPS = (2, 4, 7, 9, 12, 14)
# groups whose final runs on VectorE (the last-consumed groups: keeps the
# drain off ScalarE's tail); all other finals run on ScalarE
V_FINAL_GROUPS = (13, 15)
PREFETCH_NS = 13000.0  # how far ahead (virtual time) loads are stamped
BUFS = {"xp": 5, "hp": 7, "op": 3, "st": 8}

# virtual-time unit costs (ns) — shape emission order only
VT = {
    "cast_v": 3900.0,   # 4 casts + 4 Sx + tau0
    "cast_s": 2600.0,   # 4 casts + tau0
    "slot_v": 2800.0,   # 4 mins + 4 counts + update
    "slot_s": 4800.0,   # 4 ACT relu (counts+update ride VectorE)
    "final_v": 2700.0,  # 4 relu finals + store issue
    "final_s": 4600.0,
}

_NC_CACHE = None


def _build_nc():
    import concourse.bacc as bacc
    import concourse.mybir as mybir
    from concourse.tile import TileContext

    f32 = mybir.dt.float32
    f16 = mybir.dt.float16
    Alu = mybir.AluOpType
    Act = mybir.ActivationFunctionType

    nc = bacc.Bacc(
        "TRN2", target_bir_lowering=False, debug=False, num_devices=N_CORES
    )
    x_ext = nc.dram_tensor("x", [ROWS_PER_CORE, K], f32, kind="ExternalInput")
    out_ext = nc.dram_tensor("out", [ROWS_PER_CORE, K], f32, kind="ExternalOutput")

    ST_NAMES = ("mx", "tau", "ntau", "sx", "acc", "cacc", "r", "c", "rcp", "stp")
    NST = len(ST_NAMES)

    with TileContext(nc) as tc:
        with (
            tc.tile_pool(name="xp", bufs=BUFS["xp"]) as xp,
            tc.tile_pool(name="hp", bufs=BUFS["hp"]) as hp,
            tc.tile_pool(name="op", bufs=BUFS["op"]) as op,
            tc.tile_pool(name="scr", bufs=1) as scr,
            tc.tile_pool(name="st", bufs=BUFS["st"]) as st,
        ):
            # engine-dedicated elementwise-output scratch (never read back)
            scrR = scr.tile([P, K], f16, tag="scrR")  # V min/Sx out
            scrC = scr.tile([P, K], f16, tag="scrC")  # V count out
            scrS = scr.tile([P, K], f16, tag="scrS", name="scrS")  # S out

            # warm the ACT table (a set containing Sign AND Relu) so the
            # one-time ~2.7us table load overlaps the first DMA
            nc.scalar.activation(
                scrS[:, :1], nc.const_aps.aps[(f32, 0.0)], Act.Sign
            )
            nc.scalar.activation(
                scrS[:, 1:2], nc.const_aps.aps[(f32, 0.0)], Act.Relu
            )

            xbs, xhs, stts = {}, {}, {}
            chunk_of = {}  # group -> (chunk_key, position)
            chunk_stt = {}  # chunk_key -> stats dict over [P, 10*ntiles]

            def emit_load(g):
                rows = slice(g * G * P, (g + 1) * G * P)
                x_dram = x_ext.ap()[rows, :].rearrange("(t p) k -> p t k", p=P)
                xb = xp.tile([P, G * K], f32, tag="xb")
                if g == 0:
                    # per-tile loads so the first cast starts ~4x sooner
                    for j in range(G):
                        nc.sync.dma_start(
                            out=xb[:, j * K : (j + 1) * K], in_=x_dram[:, j, :]
                        )
                else:
                    nc.sync.dma_start(
                        out=xb[:].rearrange("p (t k) -> p t k", t=G), in_=x_dram
                    )
                xbs[g] = xb

            def emit_cast_chunk(chunk):
                """fp16 shadow + row max for every tile of the chunk; V-owned
                chunks also get Sx; one tau0 op over the whole stats width."""
                on_s = chunk[0] in S_GROUPS
                ncols = G * len(chunk)
                st_t = st.tile([P, NST * ncols], f32, tag="st")
                stt = {
                    n: st_t[:, k * ncols : (k + 1) * ncols]
                    for k, n in enumerate(ST_NAMES)
                }
                chunk_stt[chunk[0]] = stt
                for ci, g in enumerate(chunk):
                    chunk_of[g] = (chunk[0], ci)
                    xb = xbs.pop(g)
                    xh = hp.tile([P, G * K], f16, tag="xh")
                    xhs[g] = xh
                    for j in range(G):
                        sl1 = slice(ci * G + j, ci * G + j + 1)
                        nc.vector.tensor_scalar(
                            xh[:, j * K : (j + 1) * K],
                            xb[:, j * K : (j + 1) * K],
                            0.0, None, Alu.add, Alu.max,
                            accum_out=stt["mx"][:, sl1],
                        )
                        if not on_s:
                            nc.vector.tensor_scalar(
                                scrR[:], xh[:, j * K : (j + 1) * K],
                                0.0, None, Alu.add, Alu.add,
                                accum_out=stt["sx"][:, sl1],
                            )
                # tau0 = mx - D0
                nc.vector.tensor_scalar(
                    stt["tau"][:], stt["mx"][:], -D0, None, Alu.add
                )
                if on_s:
                    # ScalarE ACT biases need ntau = -tau
                    nc.vector.tensor_scalar(
                        stt["ntau"][:], stt["tau"][:], -1.0, None, Alu.mult
                    )

            def emit_slot_v_chunk(chunk, i):
                stt = chunk_stt[chunk[0]]
                tau = stt["tau"]
                for ci, g in enumerate(chunk):
                    xh = xhs[g]
                    for j in range(G):
                        sl1 = slice(ci * G + j, ci * G + j + 1)
                        xhj = xh[:, j * K : (j + 1) * K]
                        # acc = sum(min(x, tau));  r = Sx - acc
                        nc.vector.tensor_scalar(
                            scrR[:], xhj, tau[:, sl1], None, Alu.min, Alu.add,
                            accum_out=stt["acc"][:, sl1],
                        )
                        nc.vector.tensor_scalar(
                            scrC[:], xhj, tau[:, sl1], None, Alu.is_gt, Alu.add,
                            accum_out=stt["cacc"][:, sl1],
                        )
                # ---- one update chain for the whole chunk ----
                nc.vector.tensor_tensor(
                    stt["r"][:], stt["sx"][:], stt["acc"][:], Alu.subtract
                )
                nc.vector.tensor_scalar_max(stt["c"][:], stt["cacc"][:], 1.0)
                nc.vector.reciprocal(stt["rcp"][:], stt["c"][:])
                nc.vector.scalar_tensor_tensor(
                    stt["stp"][:], stt["r"][:], -1.0, stt["rcp"][:],
                    Alu.add, Alu.mult,
                )
                nc.vector.scalar_tensor_tensor(
                    tau[:], stt["stp"][:], float(LAM[i]), tau[:],
                    Alu.mult, Alu.add,
                )

            def emit_slot_s_r(g, i):
                """ScalarE side of an S-owned slot: 4 ACT Relu r-passes."""
                stt = chunk_stt[g]
                xh = xhs[g]
                ntau = stt["ntau"]
                for j in range(G):
                    sl1 = slice(j, j + 1)
                    # r = sum(relu(x + ntau))  — tiny terms, drift-free
                    nc.scalar.activation(
                        scrS[:], xh[:, j * K : (j + 1) * K], Act.Relu,
                        bias=ntau[:, sl1], accum_out=stt["r"][:, sl1],
                    )

            def emit_slot_s_v(g, i):
                """VectorE side: is_gt counts + the update chain."""
                stt = chunk_stt[g]
                xh = xhs[g]
                tau = stt["tau"]
                for j in range(G):
                    sl1 = slice(j, j + 1)
                    nc.vector.tensor_scalar(
                        scrC[:], xh[:, j * K : (j + 1) * K], tau[:, sl1],
                        None, Alu.is_gt, Alu.add,
                        accum_out=stt["cacc"][:, sl1],
                    )
                nc.vector.tensor_scalar_max(stt["c"][:], stt["cacc"][:], 1.0)
                nc.vector.reciprocal(stt["rcp"][:], stt["c"][:])
                nc.vector.scalar_tensor_tensor(
                    stt["stp"][:], stt["r"][:], -1.0, stt["rcp"][:],
                    Alu.add, Alu.mult,
                )
                # tau += lam * stp;  ntau = -tau
                nc.vector.scalar_tensor_tensor(
                    stt["tau"][:], stt["stp"][:], float(LAM[i]), stt["tau"][:],
                    Alu.mult, Alu.add,
                )
                nc.vector.tensor_scalar(
                    stt["ntau"][:], stt["tau"][:], -1.0, None, Alu.mult
                )

            def emit_final(g):
                final_v = g in V_FINAL_GROUPS
                ckey, ci = chunk_of[g]
                stt = chunk_stt[ckey]
                xh = xhs.pop(g)
                rows = slice(g * G * P, (g + 1) * G * P)
                o_dram = out_ext.ap()[rows, :].rearrange("(t p) k -> p t k", p=P)
                ob = op.tile([P, G * K], f32, tag="ob")
                if not final_v and g not in S_GROUPS:
                    # ScalarE final for a V-owned group: materialize ntau
                    sl = slice(ci * G, (ci + 1) * G)
                    nc.vector.tensor_scalar(
                        stt["ntau"][:, sl], stt["tau"][:, sl], -1.0, None,
                        Alu.mult,
                    )
                for j in range(G):
                    sl1 = slice(ci * G + j, ci * G + j + 1)
                    xhj = xh[:, j * K : (j + 1) * K]
                    obj = ob[:, j * K : (j + 1) * K]
                    if final_v:
                        nc.vector.tensor_scalar(
                            obj, xhj, stt["tau"][:, sl1], 0.0,
                            Alu.subtract, Alu.max,
                        )
                    else:
                        nc.scalar.activation(
                            obj, xhj, Act.Relu, bias=stt["ntau"][:, sl1]
                        )
                nc.gpsimd.dma_start(
                    out=o_dram, in_=ob[:].rearrange("p (t k) -> p t k", t=G)
                )

            # ---- build the two lanes as (vtime, seq, emit_fn) events ----
            events = []
            seq = 0

            def push(t, fn):
                nonlocal seq
                events.append((t, seq, fn))
                seq += 1

            v_list = [g for g in range(N_GROUPS) if g not in S_GROUPS]
            s_list = [g for g in range(N_GROUPS) if g in S_GROUPS]
            v_chunks = [v_list[i : i + 2] for i in range(0, len(v_list), 2)]

            cast_time = {}
            vt = 0.0
            for chunk in v_chunks:
                for g in chunk:
                    cast_time[g] = vt
                push(vt, lambda c=chunk: emit_cast_chunk(c))
                vt += VT["cast_v"] * len(chunk)
                for i in range(N_SLOTS):
                    push(vt, lambda c=chunk, i=i: emit_slot_v_chunk(c, i))
                    vt += VT["slot_v"] * len(chunk)
                for g in chunk:
                    push(vt, lambda g=g: emit_final(g))
                    vt += VT["final_v"]
            v_span = vt

            # stretch the ScalarE lane across the whole run so its own
            # groups interleave evenly with the finals it also executes
            s_span = len(s_list) * (
                VT["cast_s"] + N_SLOTS * VT["slot_s"] + VT["final_s"]
            )
            stretch = max(0.9 * v_span / s_span, 1.0) if s_span else 1.0
            S_OFF = -2000.0  # S lane leads slightly so its first load is first
            vt = 0.0
            pairs = [s_list[i : i + 2] for i in range(0, len(s_list), 2)]
            for pair in pairs:
                for g in pair:
                    cast_time[g] = vt * stretch + S_OFF
                    push(vt * stretch + S_OFF, lambda g=g: emit_cast_chunk([g]))
                    vt += VT["cast_s"]
                for i in range(N_SLOTS):
                    for g in pair:
                        push(vt * stretch + S_OFF,
                             lambda g=g, i=i: emit_slot_s_r(g, i))
                        push(vt * stretch + S_OFF + 1.0,
                             lambda g=g, i=i: emit_slot_s_v(g, i))
                        vt += VT["slot_s"]
                for g in pair:
                    push(vt * stretch + S_OFF, lambda g=g: emit_final(g))
                    vt += VT["final_s"]

            for g in range(N_GROUPS):
                push(cast_time[g] - PREFETCH_NS, lambda g=g: emit_load(g))

            events.sort(key=lambda e: (e[0], e[1]))
            for _, _, fn in events:
                fn()

    nc.compile()
    return nc


def _get_nc():
    global _NC_CACHE
    if _NC_CACHE is None:
        _NC_CACHE = _build_nc()
    return _NC_CACHE


def _effective_alpha(alpha):
    # the module's ClampMin/ClampMax pair, verbatim in numpy
    a = np.asarray(alpha, dtype=np.float32)
    a = np.maximum(np.minimum(a, 0.0) - 1.0, 0.0) + 1.0 + np.maximum(a, 0.0)
    a = np.minimum(np.maximum(a, 0.0) - 2.0, 0.0) + 2.0 + np.minimum(a, 0.0)
    return a.astype(np.float32)


def _entmax_bisect_numpy(x, a, n_iter=50):
    """Generic-alpha fallback replicating the reference bisection in f32.
    Never taken for alpha in [1,2] (the clamp maps those to exactly 2.0)."""
    f32 = np.float32
    X = x.reshape(-1, K).astype(np.float32)
    am1 = (np.broadcast_to(a.reshape(1, H), (B, H)).reshape(-1)[
        np.arange(X.shape[0]) // Q
    ].astype(np.float32) - f32(1.0))[:, None]
    Xs = (X * am1).astype(np.float32)

    def p(s):
        pos = s > 0
        return np.where(
            pos, np.power(np.where(pos, s, f32(1.0)), (f32(1.0) / am1)), f32(0.0)
        ).astype(np.float32)

    mx = Xs.max(axis=1, keepdims=True).astype(np.float32)
    tau_lo = (mx - f32(1.0)).astype(np.float32)
    tau_hi = (mx - np.power(f32(1.0 / K), am1)).astype(np.float32)
    f_lo = (p(Xs - tau_lo).sum(axis=1, dtype=np.float32)[:, None] - f32(1.0)).astype(
        np.float32
    )
    dm = (tau_hi - tau_lo).astype(np.float32)
    tau_m = tau_lo.copy()
    for _ in range(n_iter):
        dm = (dm * f32(0.5)).astype(np.float32)
        tau_m = (tau_lo + dm).astype(np.float32)
        f_m = (p(Xs - tau_m).sum(axis=1, dtype=np.float32)[:, None] - f32(1.0)).astype(
            np.float32
        )
        tau_lo = np.where(f_m * f_lo >= 0, tau_m, tau_lo).astype(np.float32)
    pm = p(Xs - tau_m)
    s = pm.sum(axis=1, dtype=np.float32).astype(np.float32)[:, None]
    return (pm / s).astype(np.float32).reshape(B, H, Q, K)


def kernel(**inputs) -> np.ndarray:
    from concourse.bass_utils import run_bass_kernel_spmd

    x = np.ascontiguousarray(np.asarray(inputs["x"], dtype=np.float32))
    alpha = np.asarray(inputs.get("alpha", np.full((1, H), 1.5, np.float32)))
    a_eff = _effective_alpha(alpha)
    if not np.all(a_eff == np.float32(2.0)):
        # out-of-distribution alpha (outside [1,2]): generic slow path
        return _entmax_bisect_numpy(x, a_eff)

    shards = x.reshape(N_CORES, ROWS_PER_CORE, K)
    in_maps = [{"x": shards[i]} for i in range(N_CORES)]

    nc = _get_nc()
    res = run_bass_kernel_spmd(nc, in_maps, core_ids=list(range(N_CORES)))
    out = np.stack([res.results[i]["out"] for i in range(N_CORES)])
    return out.reshape(B, H, Q, K)


# revision 33
# speedup vs baseline: 1.1589x; 1.0096x over previous
"""Trainium2 Bass kernel for nn_AlphaEntmax (entmax-bisect over last axis).

Key math fact: the module's ClampMin/ClampMax composition maps any alpha in
[1,2] to exactly 2.0, so the reference computes sparsemax (alpha=2) per row:
    p = relu(x - tau) / sum(relu(x - tau)),  tau s.t. sum(relu(x - tau)) = 1
We solve for tau with 4 over-relaxed Newton steps from tau0 = rowmax - 0.8
(tau' = tau + lam*(r-1)/c with r = sum(relu(x-tau)), c = count(x > tau),
lam = [1.15, 1.10, 1.05, 1.0]), then emit p = relu(x - tau) from the fp16
shadow (sum==1 at convergence; the reference's own normalize brings both
within tolerance; fp16 sourcing costs ~2e-3 of the 2e-2 budget).

Architecture: OWNERSHIP LANES — each engine owns whole tiles end-to-end so
no per-slot cross-engine barrier exists (GPSIMD cannot run compute: the
neuronxcc ISA check rejects TensorScalarPtr on Pool).
  - VectorE-owned tiles (48): per slot a min-pass  acc = sum(min(x,tau))
    (tensor_scalar op0=min accum=add, fp16 4x mode ~300ns) and an is_gt
    count pass. r = Sx - acc with Sx = sum(x) computed once per tile: the
    min-sum's magnitude is ~|Sx| ~ 30, so the HW f32 accumulator drift is
    ~1e-5. (The cheaper-looking dual  sum(max(x,tau)) = K*tau + r  has
    accumulated magnitude ~2500 and drifts ~5e-2 on HW — unusable.)
  - ScalarE-owned tiles (16): ACT Relu(x + ntau, accum) gives r directly
    (relu terms are tiny, no drift) and ACT Sign(x + ntau, accum) gives
    K - 2c; iterates in ntau space. ACT is dtype-independent so it reads
    the fp16 shadow; ~1.1us per pass.
  - the fp16 (not bf16) shadow keeps the quantization floor ~8x lower at
    the same 4x DVE speed. All update chains are small [128,4] VectorE ops.
  - finals: owner engine emits relu(x_h - tau) to an f32 buffer; stores
    ride the Pool SWDGE queue, loads the SP queue (no head-of-line mixing).
  - emission: every lane unit gets a virtual-time stamp (its serial cost
    on the owning engine); units from all lanes are merged in stamp order
    so the in-order engines always interleave across tiles; loads are
    stamped ~15us before their tile's first use.
DMA is the roofline: 32MB in + 32MB out per core over ~360GB/s ≈ 186us.

Sharding: x [8,16,512,1024] is split along the batch axis, one batch entry
(8192 rows of 1024) per NeuronCore; no cross-core communication.
"""

import numpy as np

B, H, Q, K = 8, 16, 512, 1024
N_CORES = 8
P = 128
ROWS_PER_CORE = (B // N_CORES) * H * Q  # 8192
N_TILES = ROWS_PER_CORE // P  # 64
G = 4  # tiles per group (DMA/stats granularity)
N_GROUPS = N_TILES // G  # 16
N_SLOTS = 4
D0 = 0.8  # tau0 = rowmax - D0
LAM = [1.15, 1.10, 1.05, 1.0]  # per-slot Newton over-relaxation
# group indices owned by ScalarE (spread through the run for even pacing);
# processed in interleaved pairs so VectorE's count+update for one group
# overlaps ScalarE's r-passes on the other
S_GROUPS = (2, 4, 7, 9, 12, 14)
# groups whose final runs on VectorE (the last-consumed groups: keeps the
# drain off ScalarE's tail); all other finals run on ScalarE
V_FINAL_GROUPS = (13, 15)
PREFETCH_NS = 13000.0  # how far ahead (virtual time) loads are stamped
BUFS = {"xp": 5, "hp": 7, "op": 3, "st": 8}

# virtual-time unit costs (ns) — shape emission order only
VT = {
    "cast_v": 3900.0,   # 4 casts + 4 Sx + tau0
    "cast_s": 2600.0,   # 4 casts + tau0
    "slot_v": 2800.0,   # 4 mins + 4 counts + update
    "slot_s": 4800.0,   # 4 ACT relu (counts+update ride VectorE)
    "final_v": 2700.0,  # 4 relu finals + store issue
    "final_s": 4600.0,
}

_NC_CACHE = None


def _build_nc():
    import concourse.bacc as bacc
    import concourse.mybir as mybir
    from concourse.tile import TileContext

    f32 = mybir.dt.float32
    f16 = mybir.dt.float16
    Alu = mybir.AluOpType
    Act = mybir.ActivationFunctionType

    nc = bacc.Bacc(
        "TRN2", target_bir_lowering=False, debug=False, num_devices=N_CORES
    )
    x_ext = nc.dram_tensor("x", [ROWS_PER_CORE, K], f32, kind="ExternalInput")
    out_ext = nc.dram_tensor("out", [ROWS_PER_CORE, K], f32, kind="ExternalOutput")

    ST_NAMES = ("mx", "tau", "ntau", "sx", "acc", "cacc", "r", "c", "rcp", "stp")
    NST = len(ST_NAMES)

    with TileContext(nc) as tc:
        with (
            tc.tile_pool(name="xp", bufs=BUFS["xp"]) as xp,
            tc.tile_pool(name="hp", bufs=BUFS["hp"]) as hp,
            tc.tile_pool(name="op", bufs=BUFS["op"]) as op,
            tc.tile_pool(name="scr", bufs=1) as scr,
            tc.tile_pool(name="st", bufs=BUFS["st"]) as st,
        ):
            # engine-dedicated elementwise-output scratch (never read back)
            scrR = scr.tile([P, K], f16, tag="scrR")  # V min/Sx out
            scrC = scr.tile([P, K], f16, tag="scrC")  # V count out
            scrS = scr.tile([P, K], f16, tag="scrS", name="scrS")  # S out

            # warm the ACT table (a set containing Sign AND Relu) so the
            # one-time ~2.7us table load overlaps the first DMA
            nc.scalar.activation(
                scrS[:, :1], nc.const_aps.aps[(f32, 0.0)], Act.Sign
            )
            nc.scalar.activation(
                scrS[:, 1:2], nc.const_aps.aps[(f32, 0.0)], Act.Relu
            )

            xbs, xhs, stts = {}, {}, {}
            chunk_of = {}  # group -> (chunk_key, position)
            chunk_stt = {}  # chunk_key -> stats dict over [P, 10*ntiles]

            def emit_load(g):
                rows = slice(g * G * P, (g + 1) * G * P)
                x_dram = x_ext.ap()[rows, :].rearrange("(t p) k -> p t k", p=P)
                xb = xp.tile([P, G * K], f32, tag="xb")
                if g == 0:
                    # per-tile loads so the first cast starts ~4x sooner
                    for j in range(G):
                        nc.sync.dma_start(
                            out=xb[:, j * K : (j + 1) * K], in_=x_dram[:, j, :]
                        )
                else:
                    nc.sync.dma_start(
                        out=xb[:].rearrange("p (t k) -> p t k", t=G), in_=x_dram
                    )
                xbs[g] = xb

            def emit_cast_chunk(chunk):
                """fp16 shadow + row max for every tile of the chunk; V-owned
                chunks also get Sx; one tau0 op over the whole stats width."""
                on_s = chunk[0] in S_GROUPS
                ncols = G * len(chunk)
                st_t = st.tile([P, NST * ncols], f32, tag="st")
                stt = {
                    n: st_t[:, k * ncols : (k + 1) * ncols]
                    for k, n in enumerate(ST_NAMES)
                }
                chunk_stt[chunk[0]] = stt
                for ci, g in enumerate(chunk):
                    chunk_of[g] = (chunk[0], ci)
                    xb = xbs.pop(g)
                    xh = hp.tile([P, G * K], f16, tag="xh")
                    xhs[g] = xh
                    for j in range(G):
                        sl1 = slice(ci * G + j, ci * G + j + 1)
                        nc.vector.tensor_scalar(
                            xh[:, j * K : (j + 1) * K],
                            xb[:, j * K : (j + 1) * K],
                            0.0, None, Alu.add, Alu.max,
                            accum_out=stt["mx"][:, sl1],
                        )
                        if not on_s:
                            nc.vector.tensor_scalar(
                                scrR[:], xh[:, j * K : (j + 1) * K],
                                0.0, None, Alu.add, Alu.add,
                                accum_out=stt["sx"][:, sl1],
                            )
                # tau0 = mx - D0
                nc.vector.tensor_scalar(
                    stt["tau"][:], stt["mx"][:], -D0, None, Alu.add
                )
                if on_s:
                    # ScalarE ACT biases need ntau = -tau
                    nc.vector.tensor_scalar(
                        stt["ntau"][:], stt["tau"][:], -1.0, None, Alu.mult
                    )

            def emit_slot_v_chunk(chunk, i):
                stt = chunk_stt[chunk[0]]
                tau = stt["tau"]
                for ci, g in enumerate(chunk):
                    xh = xhs[g]
                    for j in range(G):
                        sl1 = slice(ci * G + j, ci * G + j + 1)
                        xhj = xh[:, j * K : (j + 1) * K]
                        # acc = sum(min(x, tau));  r = Sx - acc
                        nc.vector.tensor_scalar(
                            scrR[:], xhj, tau[:, sl1], None, Alu.min, Alu.add,
                            accum_out=stt["acc"][:, sl1],
                        )
                        nc.vector.tensor_scalar(
                            scrC[:], xhj, tau[:, sl1], None, Alu.is_gt, Alu.add,
                            accum_out=stt["cacc"][:, sl1],
                        )
                # ---- one update chain for the whole chunk ----
                nc.vector.tensor_tensor(
                    stt["r"][:], stt["sx"][:], stt["acc"][:], Alu.subtract
                )
                nc.vector.tensor_scalar_max(stt["c"][:], stt["cacc"][:], 1.0)
                nc.vector.reciprocal(stt["rcp"][:], stt["c"][:])
                nc.vector.scalar_tensor_tensor(
                    stt["stp"][:], stt["r"][:], -1.0, stt["rcp"][:],
                    Alu.add, Alu.mult,
                )
                nc.vector.scalar_tensor_tensor(
                    tau[:], stt["stp"][:], float(LAM[i]), tau[:],
                    Alu.mult, Alu.add,
                )

            def emit_slot_s_r(g, i):
                """ScalarE side of an S-owned slot: 4 ACT Relu r-passes."""
                stt = chunk_stt[g]
                xh = xhs[g]
                ntau = stt["ntau"]
                for j in range(G):
                    sl1 = slice(j, j + 1)
                    # r = sum(relu(x + ntau))  — tiny terms, drift-free
                    nc.scalar.activation(
                        scrS[:], xh[:, j * K : (j + 1) * K], Act.Relu,
                        bias=ntau[:, sl1], accum_out=stt["r"][:, sl1],
                    )

            def emit_slot_s_v(g, i):
                """VectorE side: is_gt counts + the update chain."""
                stt = chunk_stt[g]
                xh = xhs[g]
                tau = stt["tau"]
                for j in range(G):
                    sl1 = slice(j, j + 1)
                    nc.vector.tensor_scalar(
                        scrC[:], xh[:, j * K : (j + 1) * K], tau[:, sl1],
                        None, Alu.is_gt, Alu.add,
                        accum_out=stt["cacc"][:, sl1],
                    )
                nc.vector.tensor_scalar_max(stt["c"][:], stt["cacc"][:], 1.0)
                nc.vector.reciprocal(stt["rcp"][:], stt["c"][:])
                nc.vector.scalar_tensor_tensor(
                    stt["stp"][:], stt["r"][:], -1.0, stt["rcp"][:],
                    Alu.add, Alu.mult,
                )
                # tau += lam * stp;  ntau = -tau
                nc.vector.scalar_tensor_tensor(
                    stt["tau"][:], stt["stp"][:], float(LAM[i]), stt["tau"][:],
                    Alu.mult, Alu.add,
                )
                nc.vector.tensor_scalar(
                    stt["ntau"][:], stt["tau"][:], -1.0, None, Alu.mult
                )

            def emit_final(g):
                final_v = g in V_FINAL_GROUPS
                ckey, ci = chunk_of[g]
                stt = chunk_stt[ckey]
                xh = xhs.pop(g)
                rows = slice(g * G * P, (g + 1) * G * P)
                o_dram = out_ext.ap()[rows, :].rearrange("(t p) k -> p t k", p=P)
                ob = op.tile([P, G * K], f32, tag="ob")
                if not final_v and g not in S_GROUPS:
                    # ScalarE final for a V-owned group: materialize ntau
                    sl = slice(ci * G, (ci + 1) * G)
                    nc.vector.tensor_scalar(
                        stt["ntau"][:, sl], stt["tau"][:, sl], -1.0, None,
                        Alu.mult,
                    )
                for j in range(G):
                    sl1 = slice(ci * G + j, ci * G + j + 1)
                    xhj = xh[:, j * K : (j + 1) * K]
                    obj = ob[:, j * K : (j + 1) * K]
                    if final_v:
                        nc.vector.tensor_scalar(
                            obj, xhj, stt["tau"][:, sl1], 0.0,
                            Alu.subtract, Alu.max,
                        )
                    else:
                        nc.scalar.activation(
                            obj, xhj, Act.Relu, bias=stt["ntau"][:, sl1]
                        )
                half = G // 2
                nc.gpsimd.dma_start(
                    out=o_dram[:, :half, :],
                    in_=ob[:, : half * K].rearrange("p (t k) -> p t k", t=half),
                )
                nc.gpsimd.dma_start(
                    out=o_dram[:, half:, :],
                    in_=ob[:, half * K :].rearrange("p (t k) -> p t k", t=half),
                )

            # ---- build the two lanes as (vtime, seq, emit_fn) events ----
            events = []
            seq = 0

            def push(t, fn):
                nonlocal seq
                events.append((t, seq, fn))
                seq += 1

            v_list = [g for g in range(N_GROUPS) if g not in S_GROUPS]
            s_list = [g for g in range(N_GROUPS) if g in S_GROUPS]
            v_chunks = [v_list[i : i + 2] for i in range(0, len(v_list), 2)]

            cast_time = {}
            vt = 0.0
            for chunk in v_chunks:
                for g in chunk:
                    cast_time[g] = vt
                push(vt, lambda c=chunk: emit_cast_chunk(c))
                vt += VT["cast_v"] * len(chunk)
                for i in range(N_SLOTS):
                    push(vt, lambda c=chunk, i=i: emit_slot_v_chunk(c, i))
                    vt += VT["slot_v"] * len(chunk)
                for g in chunk:
                    push(vt, lambda g=g: emit_final(g))
                    vt += VT["final_v"]
            v_span = vt

            # stretch the ScalarE lane across the whole run so its own
            # groups interleave evenly with the finals it also executes
            s_span = len(s_list) * (
                VT["cast_s"] + N_SLOTS * VT["slot_s"] + VT["final_s"]
            )
            stretch = max(0.9 * v_span / s_span, 1.0) if s_span else 1.0
            S_OFF = 1500.0  # S lane trails slightly: V's first loads go first
            vt = 0.0
            pairs = [s_list[i : i + 2] for i in range(0, len(s_list), 2)]
            for pair in pairs:
                for g in pair:
                    cast_time[g] = vt * stretch + S_OFF
                    push(vt * stretch + S_OFF, lambda g=g: emit_cast_chunk([g]))
                    vt += VT["cast_s"]
                for i in range(N_SLOTS):
                    for g in pair:
                        push(vt * stretch + S_OFF,
                             lambda g=g, i=i: emit_slot_s_r(g, i))
                        push(vt * stretch + S_OFF + 1.0,
                             lambda g=g, i=i: emit_slot_s_v(g, i))
                        vt += VT["slot_s"]
                for g in pair:
                    push(vt * stretch + S_OFF, lambda g=g: emit_final(g))
                    vt += VT["final_s"]

            for g in range(N_GROUPS):
                push(cast_time[g] - PREFETCH_NS, lambda g=g: emit_load(g))

            events.sort(key=lambda e: (e[0], e[1]))
            for _, _, fn in events:
                fn()

    nc.compile()
    return nc


def _get_nc():
    global _NC_CACHE
    if _NC_CACHE is None:
        _NC_CACHE = _build_nc()
    return _NC_CACHE


def _effective_alpha(alpha):
    # the module's ClampMin/ClampMax pair, verbatim in numpy
    a = np.asarray(alpha, dtype=np.float32)
    a = np.maximum(np.minimum(a, 0.0) - 1.0, 0.0) + 1.0 + np.maximum(a, 0.0)
    a = np.minimum(np.maximum(a, 0.0) - 2.0, 0.0) + 2.0 + np.minimum(a, 0.0)
    return a.astype(np.float32)


def _entmax_bisect_numpy(x, a, n_iter=50):
    """Generic-alpha fallback replicating the reference bisection in f32.
    Never taken for alpha in [1,2] (the clamp maps those to exactly 2.0)."""
    f32 = np.float32
    X = x.reshape(-1, K).astype(np.float32)
    am1 = (np.broadcast_to(a.reshape(1, H), (B, H)).reshape(-1)[
        np.arange(X.shape[0]) // Q
    ].astype(np.float32) - f32(1.0))[:, None]
    Xs = (X * am1).astype(np.float32)

    def p(s):
        pos = s > 0
        return np.where(
            pos, np.power(np.where(pos, s, f32(1.0)), (f32(1.0) / am1)), f32(0.0)
        ).astype(np.float32)

    mx = Xs.max(axis=1, keepdims=True).astype(np.float32)
    tau_lo = (mx - f32(1.0)).astype(np.float32)
    tau_hi = (mx - np.power(f32(1.0 / K), am1)).astype(np.float32)
    f_lo = (p(Xs - tau_lo).sum(axis=1, dtype=np.float32)[:, None] - f32(1.0)).astype(
        np.float32
    )
    dm = (tau_hi - tau_lo).astype(np.float32)
    tau_m = tau_lo.copy()
    for _ in range(n_iter):
        dm = (dm * f32(0.5)).astype(np.float32)
        tau_m = (tau_lo + dm).astype(np.float32)
        f_m = (p(Xs - tau_m).sum(axis=1, dtype=np.float32)[:, None] - f32(1.0)).astype(
            np.float32
        )
        tau_lo = np.where(f_m * f_lo >= 0, tau_m, tau_lo).astype(np.float32)
    pm = p(Xs - tau_m)
    s = pm.sum(axis=1, dtype=np.float32).astype(np.float32)[:, None]
    return (pm / s).astype(np.float32).reshape(B, H, Q, K)


def kernel(**inputs) -> np.ndarray:
    from concourse.bass_utils import run_bass_kernel_spmd

    x = np.ascontiguousarray(np.asarray(inputs["x"], dtype=np.float32))
    alpha = np.asarray(inputs.get("alpha", np.full((1, H), 1.5, np.float32)))
    a_eff = _effective_alpha(alpha)
    if not np.all(a_eff == np.float32(2.0)):
        # out-of-distribution alpha (outside [1,2]): generic slow path
        return _entmax_bisect_numpy(x, a_eff)

    shards = x.reshape(N_CORES, ROWS_PER_CORE, K)
    in_maps = [{"x": shards[i]} for i in range(N_CORES)]

    nc = _get_nc()
    res = run_bass_kernel_spmd(nc, in_maps, core_ids=list(range(N_CORES)))
    out = np.stack([res.results[i]["out"] for i in range(N_CORES)])
    return out.reshape(B, H, Q, K)


# revision 35
# speedup vs baseline: 1.1717x; 1.0111x over previous
"""Trainium2 Bass kernel for nn_AlphaEntmax (entmax-bisect over last axis).

Key math fact: the module's ClampMin/ClampMax composition maps any alpha in
[1,2] to exactly 2.0, so the reference computes sparsemax (alpha=2) per row:
    p = relu(x - tau) / sum(relu(x - tau)),  tau s.t. sum(relu(x - tau)) = 1
We solve for tau with 4 over-relaxed Newton steps from tau0 = rowmax - 0.8
(tau' = tau + lam*(r-1)/c with r = sum(relu(x-tau)), c = count(x > tau),
lam = [1.15, 1.10, 1.05, 1.0]), then emit p = relu(x - tau) from the fp16
shadow (sum==1 at convergence; the reference's own normalize brings both
within tolerance; fp16 sourcing costs ~2e-3 of the 2e-2 budget).

Architecture: OWNERSHIP LANES — each engine owns whole tiles end-to-end so
no per-slot cross-engine barrier exists (GPSIMD cannot run compute: the
neuronxcc ISA check rejects TensorScalarPtr on Pool).
  - VectorE-owned tiles (48): per slot a min-pass  acc = sum(min(x,tau))
    (tensor_scalar op0=min accum=add, fp16 4x mode ~300ns) and an is_gt
    count pass. r = Sx - acc with Sx = sum(x) computed once per tile: the
    min-sum's magnitude is ~|Sx| ~ 30, so the HW f32 accumulator drift is
    ~1e-5. (The cheaper-looking dual  sum(max(x,tau)) = K*tau + r  has
    accumulated magnitude ~2500 and drifts ~5e-2 on HW — unusable.)
  - ScalarE-owned tiles (16): ACT Relu(x + ntau, accum) gives r directly
    (relu terms are tiny, no drift) and ACT Sign(x + ntau, accum) gives
    K - 2c; iterates in ntau space. ACT is dtype-independent so it reads
    the fp16 shadow; ~1.1us per pass.
  - the fp16 (not bf16) shadow keeps the quantization floor ~8x lower at
    the same 4x DVE speed. All update chains are small [128,4] VectorE ops.
  - finals: owner engine emits relu(x_h - tau) to an f32 buffer; stores
    ride the Pool SWDGE queue, loads the SP queue (no head-of-line mixing).
  - emission: every lane unit gets a virtual-time stamp (its serial cost
    on the owning engine); units from all lanes are merged in stamp order
    so the in-order engines always interleave across tiles; loads are
    stamped ~15us before their tile's first use.
DMA is the roofline: 32MB in + 32MB out per core over ~360GB/s ≈ 186us.

Sharding: x [8,16,512,1024] is split along the batch axis, one batch entry
(8192 rows of 1024) per NeuronCore; no cross-core communication.
"""

import numpy as np

B, H, Q, K = 8, 16, 512, 1024
N_CORES = 8
P = 128
ROWS_PER_CORE = (B // N_CORES) * H * Q  # 8192
N_TILES = ROWS_PER_CORE // P  # 64
G = 4  # tiles per group (DMA/stats granularity)
N_GROUPS = N_TILES // G  # 16
N_SLOTS = 4
D0 = 0.8  # tau0 = rowmax - D0
LAM = [1.15, 1.10, 1.05, 1.0]  # per-slot Newton over-relaxation
# group indices owned by ScalarE (spread through the run for even pacing);
# processed in interleaved pairs so VectorE's count+update for one group
# overlaps ScalarE's r-passes on the other
S_GROUPS = (2, 4, 7, 9, 12, 14)
# groups whose final runs on VectorE (the last-consumed groups: keeps the
# drain off ScalarE's tail); all other finals run on ScalarE
V_FINAL_GROUPS = (13, 15)
PREFETCH_NS = 13000.0  # how far ahead (virtual time) loads are stamped
BUFS = {"xp": 5, "hp": 7, "op": 3, "st": 8}

# virtual-time unit costs (ns) — shape emission order only
VT = {
    "cast_v": 3900.0,   # 4 casts + 4 Sx + tau0
    "cast_s": 2600.0,   # 4 casts + tau0
    "slot_v": 2800.0,   # 4 mins + 4 counts + update
    "slot_s": 4800.0,   # 4 ACT relu (counts+update ride VectorE)
    "final_v": 2700.0,  # 4 relu finals + store issue
    "final_s": 4600.0,
}

_NC_CACHE = None


def _build_nc():
    import concourse.bacc as bacc
    import concourse.mybir as mybir
    from concourse.tile import TileContext

    f32 = mybir.dt.float32
    f16 = mybir.dt.float16
    Alu = mybir.AluOpType
    Act = mybir.ActivationFunctionType

    nc = bacc.Bacc(
        "TRN2", target_bir_lowering=False, debug=False, num_devices=N_CORES
    )
    x_ext = nc.dram_tensor("x", [ROWS_PER_CORE, K], f32, kind="ExternalInput")
    out_ext = nc.dram_tensor("out", [ROWS_PER_CORE, K], f32, kind="ExternalOutput")

    ST_NAMES = ("mx", "tau", "ntau", "sx", "acc", "cacc", "r", "c", "rcp", "stp")
    NST = len(ST_NAMES)

    with TileContext(nc) as tc:
        with (
            tc.tile_pool(name="xp", bufs=BUFS["xp"]) as xp,
            tc.tile_pool(name="hp", bufs=BUFS["hp"]) as hp,
            tc.tile_pool(name="op", bufs=BUFS["op"]) as op,
            tc.tile_pool(name="scr", bufs=1) as scr,
            tc.tile_pool(name="st", bufs=BUFS["st"]) as st,
        ):
            # engine-dedicated elementwise-output scratch (never read back)
            scrR = scr.tile([P, K], f16, tag="scrR")  # V min/Sx out
            scrC = scr.tile([P, K], f16, tag="scrC")  # V count out
            scrS = scr.tile([P, K], f16, tag="scrS", name="scrS")  # S out

            # warm the ACT table (a set containing Sign AND Relu) so the
            # one-time ~2.7us table load overlaps the first DMA
            nc.scalar.activation(
                scrS[:, :1], nc.const_aps.aps[(f32, 0.0)], Act.Sign
            )
            nc.scalar.activation(
                scrS[:, 1:2], nc.const_aps.aps[(f32, 0.0)], Act.Relu
            )

            xbs, xhs, stts = {}, {}, {}
            chunk_of = {}  # group -> (chunk_key, position)
            chunk_stt = {}  # chunk_key -> stats dict over [P, 10*ntiles]

            def emit_load(g):
                rows = slice(g * G * P, (g + 1) * G * P)
                x_dram = x_ext.ap()[rows, :].rearrange("(t p) k -> p t k", p=P)
                xb = xp.tile([P, G * K], f32, tag="xb")
                if g <= 1:
                    # per-tile loads so the first casts start ~4x sooner
                    for j in range(G):
                        nc.sync.dma_start(
                            out=xb[:, j * K : (j + 1) * K], in_=x_dram[:, j, :]
                        )
                else:
                    nc.sync.dma_start(
                        out=xb[:].rearrange("p (t k) -> p t k", t=G), in_=x_dram
                    )
                xbs[g] = xb

            def emit_cast_chunk(chunk):
                """fp16 shadow + row max for every tile of the chunk; V-owned
                chunks also get Sx; one tau0 op over the whole stats width."""
                on_s = chunk[0] in S_GROUPS
                ncols = G * len(chunk)
                st_t = st.tile([P, NST * ncols], f32, tag="st")
                stt = {
                    n: st_t[:, k * ncols : (k + 1) * ncols]
                    for k, n in enumerate(ST_NAMES)
                }
                chunk_stt[chunk[0]] = stt
                for ci, g in enumerate(chunk):
                    chunk_of[g] = (chunk[0], ci)
                    xb = xbs.pop(g)
                    xh = hp.tile([P, G * K], f16, tag="xh")
                    xhs[g] = xh
                    for j in range(G):
                        sl1 = slice(ci * G + j, ci * G + j + 1)
                        nc.vector.tensor_scalar(
                            xh[:, j * K : (j + 1) * K],
                            xb[:, j * K : (j + 1) * K],
                            0.0, None, Alu.add, Alu.max,
                            accum_out=stt["mx"][:, sl1],
                        )
                        if not on_s:
                            nc.vector.tensor_scalar(
                                scrR[:], xh[:, j * K : (j + 1) * K],
                                0.0, None, Alu.add, Alu.add,
                                accum_out=stt["sx"][:, sl1],
                            )
                # tau0 = mx - D0
                nc.vector.tensor_scalar(
                    stt["tau"][:], stt["mx"][:], -D0, None, Alu.add
                )
                if on_s:
                    # ScalarE ACT biases need ntau = -tau
                    nc.vector.tensor_scalar(
                        stt["ntau"][:], stt["tau"][:], -1.0, None, Alu.mult
                    )

            def emit_slot_v_chunk(chunk, i):
                stt = chunk_stt[chunk[0]]
                tau = stt["tau"]
                for ci, g in enumerate(chunk):
                    xh = xhs[g]
                    for j in range(G):
                        sl1 = slice(ci * G + j, ci * G + j + 1)
                        xhj = xh[:, j * K : (j + 1) * K]
                        # acc = sum(min(x, tau));  r = Sx - acc
                        nc.vector.tensor_scalar(
                            scrR[:], xhj, tau[:, sl1], None, Alu.min, Alu.add,
                            accum_out=stt["acc"][:, sl1],
                        )
                        nc.vector.tensor_scalar(
                            scrC[:], xhj, tau[:, sl1], None, Alu.is_gt, Alu.add,
                            accum_out=stt["cacc"][:, sl1],
                        )
                # ---- one update chain for the whole chunk ----
                nc.vector.tensor_tensor(
                    stt["r"][:], stt["sx"][:], stt["acc"][:], Alu.subtract
                )
                nc.vector.tensor_scalar_max(stt["c"][:], stt["cacc"][:], 1.0)
                nc.vector.reciprocal(stt["rcp"][:], stt["c"][:])
                nc.vector.scalar_tensor_tensor(
                    stt["stp"][:], stt["r"][:], -1.0, stt["rcp"][:],
                    Alu.add, Alu.mult,
                )
                nc.vector.scalar_tensor_tensor(
                    tau[:], stt["stp"][:], float(LAM[i]), tau[:],
                    Alu.mult, Alu.add,
                )

            def emit_slot_s_r(g, i):
                """ScalarE side of an S-owned slot: 4 ACT Relu r-passes."""
                stt = chunk_stt[g]
                xh = xhs[g]
                ntau = stt["ntau"]
                for j in range(G):
                    sl1 = slice(j, j + 1)
                    # r = sum(relu(x + ntau))  — tiny terms, drift-free
                    nc.scalar.activation(
                        scrS[:], xh[:, j * K : (j + 1) * K], Act.Relu,
                        bias=ntau[:, sl1], accum_out=stt["r"][:, sl1],
                    )

            def emit_slot_s_v(g, i):
                """VectorE side: is_gt counts + the update chain."""
                stt = chunk_stt[g]
                xh = xhs[g]
                tau = stt["tau"]
                for j in range(G):
                    sl1 = slice(j, j + 1)
                    nc.vector.tensor_scalar(
                        scrC[:], xh[:, j * K : (j + 1) * K], tau[:, sl1],
                        None, Alu.is_gt, Alu.add,
                        accum_out=stt["cacc"][:, sl1],
                    )
                nc.vector.tensor_scalar_max(stt["c"][:], stt["cacc"][:], 1.0)
                nc.vector.reciprocal(stt["rcp"][:], stt["c"][:])
                nc.vector.scalar_tensor_tensor(
                    stt["stp"][:], stt["r"][:], -1.0, stt["rcp"][:],
                    Alu.add, Alu.mult,
                )
                # tau += lam * stp;  ntau = -tau
                nc.vector.scalar_tensor_tensor(
                    stt["tau"][:], stt["stp"][:], float(LAM[i]), stt["tau"][:],
                    Alu.mult, Alu.add,
                )
                nc.vector.tensor_scalar(
                    stt["ntau"][:], stt["tau"][:], -1.0, None, Alu.mult
                )

            def emit_final(g):
                final_v = g in V_FINAL_GROUPS
                ckey, ci = chunk_of[g]
                stt = chunk_stt[ckey]
                xh = xhs.pop(g)
                rows = slice(g * G * P, (g + 1) * G * P)
                o_dram = out_ext.ap()[rows, :].rearrange("(t p) k -> p t k", p=P)
                ob = op.tile([P, G * K], f32, tag="ob")
                if not final_v and g not in S_GROUPS:
                    # ScalarE final for a V-owned group: materialize ntau
                    sl = slice(ci * G, (ci + 1) * G)
                    nc.vector.tensor_scalar(
                        stt["ntau"][:, sl], stt["tau"][:, sl], -1.0, None,
                        Alu.mult,
                    )
                for j in range(G):
                    sl1 = slice(ci * G + j, ci * G + j + 1)
                    xhj = xh[:, j * K : (j + 1) * K]
                    obj = ob[:, j * K : (j + 1) * K]
                    if final_v:
                        nc.vector.tensor_scalar(
                            obj, xhj, stt["tau"][:, sl1], 0.0,
                            Alu.subtract, Alu.max,
                        )
                        # tail groups: store each tile right behind its final
                        nc.gpsimd.dma_start(
                            out=o_dram[:, j : j + 1, :],
                            in_=ob[:, j * K : (j + 1) * K].rearrange(
                                "p (t k) -> p t k", t=1
                            ),
                        )
                    else:
                        nc.scalar.activation(
                            obj, xhj, Act.Relu, bias=stt["ntau"][:, sl1]
                        )
                if not final_v:
                    half = G // 2
                    nc.gpsimd.dma_start(
                        out=o_dram[:, :half, :],
                        in_=ob[:, : half * K].rearrange("p (t k) -> p t k", t=half),
                    )
                    nc.gpsimd.dma_start(
                        out=o_dram[:, half:, :],
                        in_=ob[:, half * K :].rearrange("p (t k) -> p t k", t=half),
                    )

            # ---- build the two lanes as (vtime, seq, emit_fn) events ----
            events = []
            seq = 0

            def push(t, fn):
                nonlocal seq
                events.append((t, seq, fn))
                seq += 1

            v_list = [g for g in range(N_GROUPS) if g not in S_GROUPS]
            s_list = [g for g in range(N_GROUPS) if g in S_GROUPS]
            body = v_list[:-1]
            v_chunks = [body[i : i + 2] for i in range(0, len(body), 2)]
            v_chunks.append([v_list[-1]])  # lone tail group: shortest drain

            cast_time = {}
            vt = 0.0
            for chunk in v_chunks:
                for g in chunk:
                    cast_time[g] = vt
                push(vt, lambda c=chunk: emit_cast_chunk(c))
                vt += VT["cast_v"] * len(chunk)
                for i in range(N_SLOTS):
                    push(vt, lambda c=chunk, i=i: emit_slot_v_chunk(c, i))
                    vt += VT["slot_v"] * len(chunk)
                for g in chunk:
                    push(vt, lambda g=g: emit_final(g))
                    vt += VT["final_v"]
            v_span = vt

            # stretch the ScalarE lane across the whole run so its own
            # groups interleave evenly with the finals it also executes
            s_span = len(s_list) * (
                VT["cast_s"] + N_SLOTS * VT["slot_s"] + VT["final_s"]
            )
            stretch = max(0.9 * v_span / s_span, 1.0) if s_span else 1.0
            S_OFF = 1500.0  # S lane trails slightly: V's first loads go first
            vt = 0.0
            pairs = [s_list[i : i + 2] for i in range(0, len(s_list), 2)]
            for pair in pairs:
                for g in pair:
                    cast_time[g] = vt * stretch + S_OFF
                    push(vt * stretch + S_OFF, lambda g=g: emit_cast_chunk([g]))
                    vt += VT["cast_s"]
                for i in range(N_SLOTS):
                    for g in pair:
                        push(vt * stretch + S_OFF,
                             lambda g=g, i=i: emit_slot_s_r(g, i))
                        push(vt * stretch + S_OFF + 1.0,
                             lambda g=g, i=i: emit_slot_s_v(g, i))
                        vt += VT["slot_s"]
                for g in pair:
                    push(vt * stretch + S_OFF, lambda g=g: emit_final(g))
                    vt += VT["final_s"]

            for g in range(N_GROUPS):
                push(cast_time[g] - PREFETCH_NS, lambda g=g: emit_load(g))

            events.sort(key=lambda e: (e[0], e[1]))
            for _, _, fn in events:
                fn()

    nc.compile()
    return nc


def _get_nc():
    global _NC_CACHE
    if _NC_CACHE is None:
        _NC_CACHE = _build_nc()
    return _NC_CACHE


def _effective_alpha(alpha):
    # the module's ClampMin/ClampMax pair, verbatim in numpy
    a = np.asarray(alpha, dtype=np.float32)
    a = np.maximum(np.minimum(a, 0.0) - 1.0, 0.0) + 1.0 + np.maximum(a, 0.0)
    a = np.minimum(np.maximum(a, 0.0) - 2.0, 0.0) + 2.0 + np.minimum(a, 0.0)
    return a.astype(np.float32)


def _entmax_bisect_numpy(x, a, n_iter=50):
    """Generic-alpha fallback replicating the reference bisection in f32.
    Never taken for alpha in [1,2] (the clamp maps those to exactly 2.0)."""
    f32 = np.float32
    X = x.reshape(-1, K).astype(np.float32)
    am1 = (np.broadcast_to(a.reshape(1, H), (B, H)).reshape(-1)[
        np.arange(X.shape[0]) // Q
    ].astype(np.float32) - f32(1.0))[:, None]
    Xs = (X * am1).astype(np.float32)

    def p(s):
        pos = s > 0
        return np.where(
            pos, np.power(np.where(pos, s, f32(1.0)), (f32(1.0) / am1)), f32(0.0)
        ).astype(np.float32)

    mx = Xs.max(axis=1, keepdims=True).astype(np.float32)
    tau_lo = (mx - f32(1.0)).astype(np.float32)
    tau_hi = (mx - np.power(f32(1.0 / K), am1)).astype(np.float32)
    f_lo = (p(Xs - tau_lo).sum(axis=1, dtype=np.float32)[:, None] - f32(1.0)).astype(
        np.float32
    )
    dm = (tau_hi - tau_lo).astype(np.float32)
    tau_m = tau_lo.copy()
    for _ in range(n_iter):
        dm = (dm * f32(0.5)).astype(np.float32)
        tau_m = (tau_lo + dm).astype(np.float32)
        f_m = (p(Xs - tau_m).sum(axis=1, dtype=np.float32)[:, None] - f32(1.0)).astype(
            np.float32
        )
        tau_lo = np.where(f_m * f_lo >= 0, tau_m, tau_lo).astype(np.float32)
    pm = p(Xs - tau_m)
    s = pm.sum(axis=1, dtype=np.float32).astype(np.float32)[:, None]
    return (pm / s).astype(np.float32).reshape(B, H, Q, K)


def kernel(**inputs) -> np.ndarray:
    from concourse.bass_utils import run_bass_kernel_spmd

    x = np.ascontiguousarray(np.asarray(inputs["x"], dtype=np.float32))
    alpha = np.asarray(inputs.get("alpha", np.full((1, H), 1.5, np.float32)))
    a_eff = _effective_alpha(alpha)
    if not np.all(a_eff == np.float32(2.0)):
        # out-of-distribution alpha (outside [1,2]): generic slow path
        return _entmax_bisect_numpy(x, a_eff)

    shards = x.reshape(N_CORES, ROWS_PER_CORE, K)
    in_maps = [{"x": shards[i]} for i in range(N_CORES)]

    nc = _get_nc()
    res = run_bass_kernel_spmd(nc, in_maps, core_ids=list(range(N_CORES)))
    out = np.stack([res.results[i]["out"] for i in range(N_CORES)])
    return out.reshape(B, H, Q, K)
